# revision 1
# baseline (speedup 1.0000x reference)
"""Shared planner + Bass builder for the hypergraph conv kernel.

Parameterized by Cfg so a scaled-down version can run in MultiCoreSim.
kernel.py inlines/imports this for the full-size problem.
"""
import numpy as np
from dataclasses import dataclass, field
from collections import Counter

import concourse.bass as bass
import concourse.mybir as mybir
import concourse.bacc as bacc
import concourse.tile as tile
import bass_rust
from concourse.library_config import mlp as mlp_library
from concourse._compat import get_trn_type, cdiv

F32 = mybir.dt.float32
I16 = mybir.dt.int16
AX = mybir.AxisListType
ALU = mybir.AluOpType
ACTF = mybir.ActivationFunctionType


@dataclass
class Cfg:
    N: int = 100000
    E: int = 25000
    D: int = 128      # node/edge in dim
    H: int = 4
    C: int = 32
    NC: int = 8
    TILE_COLS: int = 30
    NSTAGE: int = 14

    @property
    def NSH(self):
        return self.N // self.NC

    @property
    def NT_ROWS(self):   # node-feature table rows (incl dummy)
        return cdiv(self.NSH + 1, 128) * 128

    @property
    def ET_ROWS(self):   # edge table rows (incl junk)
        return cdiv(self.E + 1, 128) * 128

    @property
    def DUMMY_NODE(self):
        return self.NSH

    @property
    def JUNK_EDGE(self):
        return self.E


def _runs(keys):
    if len(keys) == 0:
        return (np.zeros(0, np.int64),) * 3
    change = np.flatnonzero(np.diff(keys)) + 1
    starts = np.concatenate([[0], change]).astype(np.int64)
    ends = np.concatenate([change, [len(keys)]]).astype(np.int64)
    return starts, ends - starts, keys[starts].astype(np.int64)


@dataclass
class Sched:
    batches: list          # [(k, tile, c0)]
    groups: list           # [(tile, c0, k, B, b0, chunk)]
    ntiles: int
    nchunks: int
    nbatch: int
    chunk_sizes: list      # batches per chunk


def _mk_schedule(lens_list, cfg: Cfg) -> Sched:
    # Sorted-ceiling batching: each core sorts its runs by length desc;
    # batch b holds that core's runs [b*128,(b+1)*128). Common batch width
    # L[b] = max over cores of the longest run in that batch. Short runs
    # inside a batch pad with dummy gather slots (contribute zero).
    sorted_lens = [np.sort(np.asarray(l))[::-1] for l in lens_list]
    nbatch_total = max(cdiv(len(l), 128) for l in sorted_lens)
    batches = []
    for b in range(nbatch_total):
        w = 1
        for ls in sorted_lens:
            if b * 128 < len(ls):
                w = max(w, int(ls[b * 128]))
        batches.append(w)
    assert max(batches) <= cfg.TILE_COLS, \
        f"run length {max(batches)} > TILE_COLS"
    placed = []
    t, c = 0, 0
    for k in batches:
        if c + k > cfg.TILE_COLS:
            t += 1
            c = 0
        placed.append((k, t, c))
        c += k
    ntiles = t + 1 if placed else 1
    nbatch = len(placed)
    nchunks = cdiv(nbatch, cfg.NSTAGE)
    chunk_sizes = [min(cfg.NSTAGE, nbatch - i * cfg.NSTAGE) for i in range(nchunks)]
    # merge consecutive batches with same (tile, k, chunk) into groups
    groups = []
    for bi, (k, t, c0) in enumerate(placed):
        ch = bi // cfg.NSTAGE
        if groups and groups[-1][0] == t and groups[-1][2] == k \
                and groups[-1][5] == ch \
                and groups[-1][1] + groups[-1][2] * groups[-1][3] == c0 \
                and groups[-1][4] + groups[-1][3] == bi:
            t0, c0g, kg, B, b0, chg = groups[-1]
            groups[-1] = (t0, c0g, kg, B + 1, b0, chg)
        else:
            groups.append((t, c0, k, 1, bi, ch))
    return Sched(placed, groups, ntiles, nchunks, nbatch, chunk_sizes)


def _wrap16(flat):
    """int64 flat stream (len % 16 == 0) -> [128, len/16] int16 wrapped+replicated."""
    assert len(flat) % 16 == 0
    b = flat.reshape(-1, 16).T.astype(np.int16)   # [16, S]
    return np.tile(b, (8, 1))                     # [128, S]


# SDMA packet ceiling is 64 descriptors -> big gather/scatter instructions
# must use single_packet=False (each descriptor its own packet). With that,
# the per-instruction cap is the SWDGE ring (~1024 descs/engine); SUBMAX=40
# columns (5120 idxs, ~641 descs/engine) stays comfortably under it.
SUBMAX = 15


def _subcols(n):
    return [(i, min(SUBMAX, n - i)) for i in range(0, n, SUBMAX)]


def _mk_streams(sched: Sched, starts, lens, gvals, svals, runvals,
                dummy_g, junk_s, dummy_run, cfg: Cfg):
    """Build gather / scatter / run-gather index streams for one core+pass."""
    TC = cfg.TILE_COLS
    g_arr = np.full((sched.ntiles, TC, 128), dummy_g, np.int64)
    s_arr = np.full((sched.nbatch, 128), junk_s, np.int64)
    r_arr = np.full((sched.nbatch, 128), dummy_run, np.int64)
    order = np.argsort(-lens, kind="stable") if len(lens) else np.zeros(0, np.int64)
    for bi, (k, t, c0) in enumerate(sched.batches):
        idxs = order[bi * 128:(bi + 1) * 128]
        nr = len(idxs)
        if nr:
            st = starts[idxs]
            kr = lens[idxs]                  # per-run lengths, desc sorted
            assert kr[0] <= k
            for kk in np.unique(kr):
                sel = np.flatnonzero(kr == kk)
                gm = gvals[st[sel][None, :] + np.arange(kk)[:, None]]  # [kk, nsel]
                # int t + slice + array sel -> advanced dims lead: need [nsel, kk]
                g_arr[t, c0:c0 + kk, sel] = gm.T
            s_arr[bi, :nr] = svals[idxs]
            r_arr[bi, :nr] = runvals[idxs]
    # per-instruction wrapping (gathers are split into <=SUBMAX-col subs)
    g_idx = np.concatenate(
        [_wrap16(g_arr[t, c0:c0 + cc].reshape(-1))
         for t in range(sched.ntiles) for (c0, cc) in _subcols(TC)], axis=1)
    sc_blocks = []
    off = 0
    for nb in sched.chunk_sizes:
        sc_blocks.append(_wrap16(s_arr[off:off + nb].reshape(-1)))
        off += nb
    sc_idx = np.concatenate(sc_blocks, axis=1)
    r_idx = np.concatenate(
        [_wrap16(r_arr[b0:b0 + bb].reshape(-1))
         for (b0, bb) in _subcols(sched.nbatch)], axis=1)
    return g_idx, sc_idx, r_idx


def build_plan(node_idx, edge_idx, cfg: Cfg):
    node_idx = np.asarray(node_idx).astype(np.int64)
    edge_idx = np.asarray(edge_idx).astype(np.int64)
    percore = []
    for m in range(cfg.NC):
        sel = np.flatnonzero(node_idx // cfg.NSH == m)
        nl = node_idx[sel] - m * cfg.NSH
        eg = edge_idx[sel]
        sA, lA, vA = _runs(eg)      # edge runs, edge-sorted (input order)
        oB = np.argsort(nl, kind="stable")
        nB = nl[oB]
        eB = eg[oB]
        sB, lB, vB = _runs(nB)      # node runs, node-sorted
        percore.append(dict(nl=nl, eg=eg, sA=sA, lA=lA, vA=vA,
                            eB=eB, sB=sB, lB=lB, vB=vB))
    schedA = _mk_schedule([c["lA"] for c in percore], cfg)
    schedB = _mk_schedule([c["lB"] for c in percore], cfg)
    streams = []
    for c in percore:
        gA, scA, _ = _mk_streams(
            schedA, c["sA"], c["lA"],
            gvals=c["nl"], svals=c["vA"], runvals=c["vA"],
            dummy_g=cfg.DUMMY_NODE, junk_s=cfg.JUNK_EDGE,
            dummy_run=cfg.JUNK_EDGE, cfg=cfg)
        gB, scB, rB = _mk_streams(
            schedB, c["sB"], c["lB"],
            gvals=c["eB"], svals=c["vB"], runvals=c["vB"],
            dummy_g=cfg.JUNK_EDGE, junk_s=cfg.DUMMY_NODE,
            dummy_run=cfg.DUMMY_NODE, cfg=cfg)
        streams.append(dict(gA=gA, scA=scA, gB=gB, scB=scB, rB=rB))
    return schedA, schedB, streams


def _ap(t_ap, off, dims):
    """Custom AP view: t_ap base AP (tile[:]), off in elements into free dim,
    dims = [[step,count],...] for free dims; partition dim kept from base."""
    base = t_ap
    part = base.ap[0]
    return bass_rust.AP(base.tensor, base.offset + off, [part] + dims)


def build_bass(cfg: Cfg, schedA: Sched, schedB: Sched, replica_groups):
    import os
    _stops = ["init", "phase1", "passA", "coll", "ea", "full"]
    _stop = _stops.index(os.environ.get("GNN_STOP", "full"))
    _dbg = os.environ.get("GNN_DEBUG_OUTS", "0") == "1"
    TC, NS = cfg.TILE_COLS, cfg.NSTAGE
    H, C = cfg.H, cfg.C
    D = cfg.D
    NT, ET = cfg.NT_ROWS, cfg.ET_ROWS
    NSH, E = cfg.NSH, cfg.E
    n_a_node = NT // 128      # a-slots for node tables
    n_a_edge = ET // 128

    nc = bacc.Bacc(get_trn_type() or "TRN2", target_bir_lowering=False, debug=False,
                   num_swdge_queues=4)
    _qrr = [0]
    def _q():
        _qrr[0] += 1
        return _qrr[0] % 4

    # ---- I/O ----
    xT = nc.dram_tensor("xT", [D, NT], F32, kind="ExternalInput")
    haT = nc.dram_tensor("haT", [D, ET], F32, kind="ExternalInput")
    Wn = nc.dram_tensor("Wn", [D, H * C], F32, kind="ExternalInput")
    We = nc.dram_tensor("We", [D, H * C], F32, kind="ExternalInput")
    attn = nc.dram_tensor("attn", [128, H * C], F32, kind="ExternalInput")
    bias_t = nc.dram_tensor("bias_t", [128, 16 * H * C], F32, kind="ExternalInput")
    dummyrows = nc.dram_tensor("dummyrows", [NT - NSH, H * C], F32, kind="ExternalInput")
    gA_i = nc.dram_tensor("gA_i", [128, schedA.ntiles * TC * 8], I16, kind="ExternalInput")
    scA_i = nc.dram_tensor("scA_i", [128, schedA.nbatch * 8], I16, kind="ExternalInput")
    gB_i = nc.dram_tensor("gB_i", [128, schedB.ntiles * TC * 8], I16, kind="ExternalInput")
    scB_i = nc.dram_tensor("scB_i", [128, schedB.nbatch * 8], I16, kind="ExternalInput")
    rB_i = nc.dram_tensor("rB_i", [128, schedB.nbatch * 8], I16, kind="ExternalInput")
    y = nc.dram_tensor("y", [NT, H * C], F32, kind="ExternalOutput")
    if _dbg:
        nf_dbg = nc.dram_tensor("nf_dbg", [NT, 128], F32, kind="ExternalOutput")
        expl_dbg = nc.dram_tensor("expl_dbg", [NT, 64], F32, kind="ExternalOutput")
        U_dbg = nc.dram_tensor("U_dbg", [ET, 128], F32, kind="ExternalOutput")
        D_dbg = nc.dram_tensor("D_dbg", [ET, 64], F32, kind="ExternalOutput")
        EAp_dbg = nc.dram_tensor("EAp_dbg", [ET, 128], F32, kind="ExternalOutput")

    # ---- internal DRAM ----
    nf_table = nc.dram_tensor("nf_table", [NT, 128], F32)
    expl_table = nc.dram_tensor("expl_table", [NT, 64], F32)
    ef_table = nc.dram_tensor("ef_table", [ET, 128], F32)
    U_table = nc.dram_tensor("U_table", [ET, 128], F32)
    D_table = nc.dram_tensor("D_table", [ET, 64], F32)
    Dpk = nc.dram_tensor("Dpk", [128, n_a_edge * 4], F32)
    U_red = nc.dram_tensor("U_red", [ET, 128], F32, addr_space="Shared")
    D_red = nc.dram_tensor("D_red", [128, n_a_edge * 4], F32, addr_space="Shared")
    EAp = nc.dram_tensor("EAp", [ET, 128], F32)

    nf_v = nf_table[:].rearrange("(a p) c -> p a c", p=128)
    expl_v = expl_table[:].rearrange("(a p) c -> p a c", p=128)
    ef_v = ef_table[:].rearrange("(a p) c -> p a c", p=128)
    U_v = U_table[:].rearrange("(a p) c -> p a c", p=128)
    D_v = D_table[:].rearrange("(a p) c -> p a c", p=128)
    Ured_v = U_red[:].rearrange("(a p) c -> p a c", p=128)
    EAp_v = EAp[:].rearrange("(a p) c -> p a c", p=128)
    y_v = y[:].rearrange("(a p) c -> p a c", p=128)

    with tile.TileContext(nc) as tc:
        with tc.tile_pool(name="const", bufs=1) as cpool:
            nc.gpsimd.load_library(mlp_library)
            Wn_sb = cpool.tile([D, H * C], F32)
            We_sb = cpool.tile([D, H * C], F32)
            attn_sb = cpool.tile([128, H * C], F32)
            bias_sb = cpool.tile([128, 16 * H * C], F32)
            zeros_sb = cpool.tile([128, 2048], F32)
            nc.sync.dma_start(Wn_sb[:], Wn[:])
            nc.sync.dma_start(We_sb[:], We[:])
            nc.sync.dma_start(attn_sb[:], attn[:])
            nc.sync.dma_start(bias_sb[:], bias_t[:])
            nc.gpsimd.memset(zeros_sb[:], 0.0)

            # ---------- init: U=0, D=0, y=bias ----------
            zv = zeros_sb[:]
            for a0 in range(0, n_a_edge, 16):
                aa = min(16, n_a_edge - a0)
                nc.sync.dma_start(U_v[:, a0:a0 + aa, :],
                                  zv.rearrange("p (a c) -> p a c", c=128)[:, :aa, :])
            for a0 in range(0, n_a_edge, 32):
                aa = min(32, n_a_edge - a0)
                nc.sync.dma_start(D_v[:, a0:a0 + aa, :],
                                  zv.rearrange("p (a c) -> p a c", c=64)[:, :aa, :])
            bv = bias_sb[:].rearrange("p (a c) -> p a c", c=H * C)
            for a0 in range(0, n_a_node, 16):
                aa = min(16, n_a_node - a0)
                nc.sync.dma_start(y_v[:, a0:a0 + aa, :], bv[:, :aa, :])

            # ---------- phase 1: projections ----------
            if _stop >= 1:
                with (tc.tile_pool(name="p1", bufs=2) as p1,
                    tc.tile_pool(name="p1s", bufs=2) as p1s,
                    tc.tile_pool(name="ps", bufs=4, space="PSUM") as psp):
                  CHK = 16
                  # node side
                  for a0 in range(0, n_a_node, CHK):
                      aa = min(CHK, n_a_node - a0)
                      xc = p1.tile([D, CHK * 128], F32, tag="xc")
                      nc.sync.dma_start(xc[:, :aa * 128], xT[:, a0 * 128:(a0 + aa) * 128])
                      nfst = p1s.tile([128, CHK, 128], F32, tag="nfst")
                      for i in range(aa):
                          mm = psp.tile([128, 128], F32, tag="mm")
                          nc.tensor.matmul(mm[:], xc[:, i * 128:(i + 1) * 128], Wn_sb[:],
                                           start=True, stop=True)
                          nc.vector.tensor_copy(nfst[:, i, :], mm[:])
                      nc.sync.dma_start(nf_v[:, a0:a0 + aa, :], nfst[:, :aa, :])
                      # attn dot + exp -> expl staging
                      tmp1 = p1s.tile([128, CHK, 128], F32, tag="tmp1")
                      nc.vector.tensor_tensor(
                          out=tmp1[:, :aa, :].rearrange("p a (h c) -> p a h c", h=H),
                          in0=nfst[:, :aa, :].rearrange("p a (h c) -> p a h c", h=H),
                          in1=attn_sb[:].rearrange("p (h c) -> p h c", h=H)
                              .unsqueeze(1).broadcast_to([128, aa, H, C]),
                          op=ALU.mult)
                      praw = p1s.tile([128, CHK, H], F32, tag="praw")
                      nc.vector.tensor_reduce(
                          out=praw[:, :aa, :],
                          in_=tmp1[:, :aa, :].rearrange("p a (h c) -> p a h c", h=H),
                          axis=AX.X, op=ALU.add)
                      est = p1s.tile([128, CHK, 64], F32, tag="est")
                      nc.gpsimd.memset(est[:], 0.0)
                      nc.scalar.activation(est[:, :aa, 0:H], praw[:, :aa, :], ACTF.Exp)
                      nc.sync.dma_start(expl_v[:, a0:a0 + aa, :], est[:, :aa, :])
                  # dummy node rows (sign-trick) — overwrite tail rows
                  nc.gpsimd.dma_start(nf_table[NSH:NT, :], dummyrows[:])
                  # edge side
                  for a0 in range(0, n_a_edge, CHK):
                      aa = min(CHK, n_a_edge - a0)
                      hc = p1.tile([D, CHK * 128], F32, tag="xc")
                      nc.sync.dma_start(hc[:, :aa * 128], haT[:, a0 * 128:(a0 + aa) * 128])
                      efst = p1s.tile([128, CHK, 128], F32, tag="nfst")
                      for i in range(aa):
                          mm = psp.tile([128, 128], F32, tag="mm")
                          nc.tensor.matmul(mm[:], hc[:, i * 128:(i + 1) * 128], We_sb[:],
                                           start=True, stop=True)
                          nc.vector.tensor_copy(efst[:, i, :], mm[:])
                      nc.sync.dma_start(ef_v[:, a0:a0 + aa, :], efst[:, :aa, :])

            # ---------- pass A ----------
            if _stop >= 2:
                with (tc.tile_pool(name="gA", bufs=2) as gpool,
                    tc.tile_pool(name="tmpA", bufs=1) as tpool,
                    tc.tile_pool(name="pA", bufs=2) as ppool,
                    tc.tile_pool(name="stA", bufs=2) as spool,
                    tc.tile_pool(name="idxA", bufs=1) as ipool):
                  gA_sb = ipool.tile([128, schedA.ntiles * TC * 8], I16)
                  scA_sb = ipool.tile([128, schedA.nbatch * 8], I16)
                  nc.sync.dma_start(gA_sb[:], gA_i[:])
                  nc.sync.dma_start(scA_sb[:], scA_i[:])

                  groups_by_tile = {}
                  for g in schedA.groups:
                      groups_by_tile.setdefault(g[0], []).append(g)

                  cur_chunk = [0]
                  stag = {}
                  sc_off = [0]

                  def open_chunk():
                      stag["U"] = spool.tile([128, NS, 128], F32, tag="ustag", name="ustag")
                      stag["D"] = spool.tile([128, NS, 64], F32, tag="dstag", name="dstag")
                      nc.gpsimd.memset(stag["D"][:], 0.0)

                  def flush_chunk():
                      ch = cur_chunk[0]
                      nb = schedA.chunk_sizes[ch]
                      nc.gpsimd.dma_scatter_add(
                          U_table[:], stag["U"][:, :nb, :],
                          scA_sb[:, sc_off[0]:sc_off[0] + nb * 8],
                          nb * 128, nb * 128, 128, single_packet=False, queue_num=_q())
                      nc.gpsimd.dma_scatter_add(
                          D_table[:], stag["D"][:, :nb, :],
                          scA_sb[:, sc_off[0]:sc_off[0] + nb * 8],
                          nb * 128, nb * 128, 64, single_packet=False, queue_num=_q())
                      sc_off[0] += nb * 8
                      cur_chunk[0] += 1

                  open_chunk()
                  for t in range(schedA.ntiles):
                      G = gpool.tile([128, TC, 128], F32, tag="G")
                      for (c0s, cc) in _subcols(TC):
                          nc.gpsimd.dma_gather(
                              G[:, c0s:c0s + cc, :], nf_table[:],
                              gA_sb[:, t * TC * 8 + c0s * 8:
                                    t * TC * 8 + (c0s + cc) * 8],
                              cc * 128, cc * 128, 128, single_packet=False, queue_num=_q())
                      tmp = tpool.tile([128, TC, 128], F32, tag="tmp")
                      nc.vector.tensor_tensor(
                          out=tmp[:].rearrange("p j (h c) -> p j h c", h=H),
                          in0=G[:].rearrange("p j (h c) -> p j h c", h=H),
                          in1=attn_sb[:].rearrange("p (h c) -> p h c", h=H)
                              .unsqueeze(1).broadcast_to([128, TC, H, C]),
                          op=ALU.mult)
                      praw = ppool.tile([128, TC, H], F32, tag="praw")
                      nc.vector.tensor_reduce(
                          out=praw[:], in_=tmp[:].rearrange("p j (h c) -> p j h c", h=H),
                          axis=AX.X, op=ALU.add)
                      pexp = ppool.tile([128, TC, H], F32, tag="pexp")
                      nc.scalar.activation(pexp[:], praw[:], ACTF.Exp)
                      for (_, c0, k, B, b0, ch) in groups_by_tile.get(t, []):
                          if ch != cur_chunk[0]:
                              flush_chunk()
                              open_chunk()
                          bpos = b0 - ch * NS
                          # denom partials: view pexp [128, B, H, k] -> reduce k
                          nc.vector.tensor_reduce(
                              out=stag["D"][:, bpos:bpos + B, 0:H],
                              in_=_ap(pexp[:], c0 * H, [[k * H, B], [1, H], [H, k]]),
                              axis=AX.X, op=ALU.add)
                          # weighted rows: tmp = G * p  (over the group's cols)
                          nc.vector.tensor_tensor(
                              out=_ap(tmp[:], c0 * 128, [[128, B * k], [32, H], [1, C]]),
                              in0=_ap(G[:], c0 * 128, [[128, B * k], [32, H], [1, C]]),
                              in1=_ap(pexp[:], c0 * H, [[H, B * k], [1, H], [0, C]]),
                              op=ALU.mult)
                          # segment sum over k: view tmp [128, B, 128, k]
                          nc.vector.tensor_reduce(
                              out=stag["U"][:, bpos:bpos + B, :],
                              in_=_ap(tmp[:], c0 * 128, [[k * 128, B], [1, 128], [128, k]]),
                              axis=AX.X, op=ALU.add)
                  flush_chunk()

            # ---------- collectives ----------
            if _stop >= 3:
                with tc.tile_pool(name="rp", bufs=2) as rpool:
                  dst = rpool.tile([128, n_a_edge, 4], F32, bufs=1)
                  for a0 in range(0, n_a_edge, 32):
                      aa = min(32, n_a_edge - a0)
                      dchunk = rpool.tile([128, 32, 64], F32, tag="dchunk")
                      nc.sync.dma_start(dchunk[:, :aa, :], D_v[:, a0:a0 + aa, :])
                      nc.vector.tensor_copy(dst[:, a0:a0 + aa, :], dchunk[:, :aa, 0:4])
                  nc.sync.dma_start(Dpk[:], dst[:].rearrange("p a c -> p (a c)"))
                nc.gpsimd.collective_compute(
                    "AllReduce", ALU.add, replica_groups=replica_groups,
                    ins=[Dpk[:]], outs=[D_red[:]])
                nc.gpsimd.collective_compute(
                    "AllReduce", ALU.add, replica_groups=replica_groups,
                    ins=[U_table[:]], outs=[U_red[:]])

            # ---------- EA' = (U/D + ef)/D ----------
            if _stop >= 4:
                with tc.tile_pool(name="ea", bufs=2) as eap:
                  dred_sb = eap.tile([128, n_a_edge * 4], F32, bufs=1)
                  invd_sb = eap.tile([128, n_a_edge * 4], F32, bufs=1)
                  nc.sync.dma_start(dred_sb[:], D_red[:])
                  nc.vector.tensor_scalar_add(dred_sb[:], dred_sb[:], 1e-30)
                  nc.vector.reciprocal(invd_sb[:], dred_sb[:])
                  invd_v = invd_sb[:].rearrange("p (a h) -> p a h", h=4)
                  for a0 in range(0, n_a_edge, 16):
                      aa = min(16, n_a_edge - a0)
                      uc = eap.tile([128, 16, 128], F32, tag="uc")
                      efc = eap.tile([128, 16, 128], F32, tag="efc")
                      nc.sync.dma_start(uc[:, :aa, :], Ured_v[:, a0:a0 + aa, :])
                      nc.sync.dma_start(efc[:, :aa, :], ef_v[:, a0:a0 + aa, :])
                      inv_b = invd_v[:, a0:a0 + aa, :].unsqueeze(3) \
                          .broadcast_to([128, aa, H, C])
                      u4 = uc[:, :aa, :].rearrange("p a (h c) -> p a h c", h=H)
                      nc.vector.tensor_tensor(out=u4, in0=u4, in1=inv_b, op=ALU.mult)
                      nc.vector.tensor_tensor(out=uc[:, :aa, :], in0=uc[:, :aa, :],
                                              in1=efc[:, :aa, :], op=ALU.add)
                      nc.vector.tensor_tensor(out=u4, in0=u4, in1=inv_b, op=ALU.mult)
                      nc.sync.dma_start(EAp_v[:, a0:a0 + aa, :], uc[:, :aa, :])
                  # zero junk rows [E:ET]
                  nj = ET - E
                  nc.gpsimd.dma_start(EAp[E:ET, :], zeros_sb[0:nj, 0:128])

            if _dbg:
                nc.gpsimd.dma_start(nf_dbg[:], nf_table[:])
                nc.gpsimd.dma_start(expl_dbg[:], expl_table[:])
                nc.gpsimd.dma_start(U_dbg[:], U_table[:])
                nc.gpsimd.dma_start(D_dbg[:], D_table[:])
                if _stop >= 4:
                    nc.gpsimd.dma_start(EAp_dbg[:], EAp[:])

            # ---------- pass B ----------
            if _stop >= 5:
                with (tc.tile_pool(name="gB", bufs=2) as gpool,
                    tc.tile_pool(name="tmpB", bufs=1) as tpool,
                    tc.tile_pool(name="stB", bufs=2) as spool,
                    tc.tile_pool(name="idxB", bufs=1) as ipool):
                  gB_sb = ipool.tile([128, schedB.ntiles * TC * 8], I16)
                  scB_sb = ipool.tile([128, schedB.nbatch * 8], I16)
                  rB_sb = ipool.tile([128, schedB.nbatch * 8], I16)
                  nc.sync.dma_start(gB_sb[:], gB_i[:])
                  nc.sync.dma_start(scB_sb[:], scB_i[:])
                  nc.sync.dma_start(rB_sb[:], rB_i[:])
                  explg = ipool.tile([128, schedB.nbatch, 64], F32)
                  for (b0s, bb) in _subcols(schedB.nbatch):
                      nc.gpsimd.dma_gather(
                          explg[:, b0s:b0s + bb, :], expl_table[:],
                          rB_sb[:, b0s * 8:(b0s + bb) * 8],
                          bb * 128, bb * 128, 64, single_packet=False, queue_num=_q())

                  groups_by_tile = {}
                  for g in schedB.groups:
                      groups_by_tile.setdefault(g[0], []).append(g)
                  cur_chunk = [0]
                  stag = {}
                  sc_off = [0]

                  def open_chunkB():
                      stag["Y"] = spool.tile([128, NS, 128], F32, tag="ystag", name="ystag")

                  def flush_chunkB():
                      ch = cur_chunk[0]
                      nb = schedB.chunk_sizes[ch]
                      nc.gpsimd.dma_scatter_add(
                          y[:], stag["Y"][:, :nb, :],
                          scB_sb[:, sc_off[0]:sc_off[0] + nb * 8],
                          nb * 128, nb * 128, 128, single_packet=False, queue_num=_q())
                      sc_off[0] += nb * 8
                      cur_chunk[0] += 1

                  open_chunkB()
                  for t in range(schedB.ntiles):
                      G = gpool.tile([128, TC, 128], F32, tag="G")
                      for (c0s, cc) in _subcols(TC):
                          nc.gpsimd.dma_gather(
                              G[:, c0s:c0s + cc, :], EAp[:],
                              gB_sb[:, t * TC * 8 + c0s * 8:
                                    t * TC * 8 + (c0s + cc) * 8],
                              cc * 128, cc * 128, 128, single_packet=False, queue_num=_q())
                      tmp = tpool.tile([128, TC, 128], F32, tag="tmp")
                      for (_, c0, k, B, b0, ch) in groups_by_tile.get(t, []):
                          if ch != cur_chunk[0]:
                              flush_chunkB()
                              open_chunkB()
                          bpos = b0 - ch * NS
                          # tmp = EAg * expl  (expl per run=partition, per batch)
                          # ISA limit: <=3 free dims, so one mult per batch
                          for b in range(B):
                              nc.vector.tensor_tensor(
                                  out=_ap(tmp[:], (c0 + b * k) * 128,
                                          [[128, k], [32, H], [1, C]]),
                                  in0=_ap(G[:], (c0 + b * k) * 128,
                                          [[128, k], [32, H], [1, C]]),
                                  in1=_ap(explg[:], (b0 + b) * 64,
                                          [[0, k], [1, H], [0, C]]),
                                  op=ALU.mult)
                          nc.vector.tensor_reduce(
                              out=stag["Y"][:, bpos:bpos + B, :],
                              in_=_ap(tmp[:], c0 * 128, [[k * 128, B], [1, 128], [128, k]]),
                              axis=AX.X, op=ALU.add)
                  flush_chunkB()
    nc.compile()
    return nc


def host_inputs(cfg: Cfg, x, ha, W_node, W_edge, attn_l, bias, streams):
    """Build per-core in_maps. x [N,D] f32, ha [E,D] f32."""
    x = np.asarray(x, np.float32)
    ha = np.asarray(ha, np.float32)
    W_node = np.asarray(W_node, np.float32)
    W_edge = np.asarray(W_edge, np.float32)
    attn_flat = np.asarray(attn_l, np.float32).reshape(-1)          # [H*C]
    bias = np.asarray(bias, np.float32).reshape(-1)                 # [H*C]
    attn_rep = np.tile(attn_flat[None, :], (128, 1))
    bias_t = np.tile(bias[None, :], (128, 16))
    dummy = (-1e3 * np.sign(attn_flat) - 1e2).astype(np.float32)    # dot << 0
    dummyrows = np.tile(dummy[None, :], (cfg.NT_ROWS - cfg.NSH, 1))
    ha_pad = np.zeros((cfg.ET_ROWS, cfg.D), np.float32)
    ha_pad[:cfg.E] = ha
    haT = np.ascontiguousarray(ha_pad.T)
    in_maps = []
    for m in range(cfg.NC):
        xs = np.zeros((cfg.NT_ROWS, cfg.D), np.float32)
        xs[:cfg.NSH] = x[m * cfg.NSH:(m + 1) * cfg.NSH]
        st = streams[m]
        in_maps.append({
            "xT": np.ascontiguousarray(xs.T),
            "haT": haT,
            "Wn": W_node, "We": W_edge,
            "attn": attn_rep, "bias_t": bias_t,
            "dummyrows": dummyrows,
            "gA_i": st["gA"], "scA_i": st["scA"],
            "gB_i": st["gB"], "scB_i": st["scB"], "rB_i": st["rB"],
        })
    return in_maps


# ======================== public entry point ========================
_CFG = Cfg()
LAST_RESULTS = None   # BassKernelResults of the most recent run (for test.py)


def _install_axon_ntff_shim():
    """Provide antenv.axon_hooks + local-only artifact handling so that
    trace=True works under axon in this container. Only used when
    GNN_TRACE=1; the plain grading path never enters here."""
    import sys, types, ctypes, contextlib
    import concourse.bass_utils as bu
    bu.upload_artifacts = lambda d: str(d)   # zero-egress container
    try:
        from antenv.axon_hooks import get_axon_ntff_profile_hook  # noqa
        return
    except ImportError:
        pass
    so_path = "/opt/axon/libaxon_pjrt.so"
    try:
        lib = ctypes.CDLL(so_path)
    except OSError:
        return
    if not hasattr(lib, "axon_start_nrt_profile"):
        return
    lib.axon_start_nrt_profile.argtypes = [ctypes.POINTER(ctypes.c_int64),
                                           ctypes.c_size_t]
    lib.axon_start_nrt_profile.restype = ctypes.c_int64
    lib.axon_stop_nrt_profile.argtypes = [ctypes.c_char_p]
    lib.axon_stop_nrt_profile.restype = ctypes.c_int64

    @contextlib.contextmanager
    def _hook(output_dir, device_ids):
        import jax
        jax.devices()
        if device_ids:
            ids = (ctypes.c_int64 * len(device_ids))(*device_ids)
            rc = lib.axon_start_nrt_profile(ids, len(device_ids))
        else:
            rc = lib.axon_start_nrt_profile(None, 0)
        if rc != 0:
            raise RuntimeError(f"axon_start_nrt_profile rc={rc}")
        try:
            yield
        finally:
            n = lib.axon_stop_nrt_profile(str(output_dir).encode())
            print(f"ntff profile: {n} file(s) -> {output_dir}")

    mod = types.ModuleType("antenv.axon_hooks")
    mod.get_axon_ntff_profile_hook = lambda: _hook
    mod.set_axon_ntff_profile_hook = lambda h: None
    sys.modules["antenv.axon_hooks"] = mod


def kernel(**inputs) -> np.ndarray:
    import os
    from concourse.bass_utils import run_bass_kernel_spmd
    cfg = _CFG
    x = np.asarray(inputs["x"], np.float32)
    ha = np.asarray(inputs["hyperedge_attr"], np.float32)
    node_idx = np.asarray(inputs["node_idx"]).astype(np.int64)
    edge_idx = np.asarray(inputs["edge_idx"]).astype(np.int64)
    schedA, schedB, streams = build_plan(node_idx, edge_idx, cfg)
    nc = build_bass(cfg, schedA, schedB, [list(range(cfg.NC))])
    in_maps = host_inputs(cfg, x, ha, inputs["W_node"], inputs["W_edge"],
                          inputs["attn_l"], inputs["bias"], streams)
    trace = os.environ.get("GNN_TRACE", "0") == "1"
    if trace:
        _install_axon_ntff_shim()
    res = run_bass_kernel_spmd(nc, in_maps, list(range(cfg.NC)), trace=trace)
    global LAST_RESULTS
    LAST_RESULTS = res
    out = np.concatenate(
        [np.asarray(res.results[m]["y"])[:cfg.NSH] for m in range(cfg.NC)], axis=0)
    return np.ascontiguousarray(out, dtype=np.float32)



# revision 7
# speedup vs baseline: 1.3780x; 1.3780x over previous
"""Hypergraph conv kernel for TRN2 (8 NeuronCores), v2.

Design (per core, incidences sharded by node id):
- phase 1: node projection nf = x@Wn, w = exp((nf*attn).sum per head);
  fused bf16 gather rows [w*nf (128ch) | w (4) | pad] -> nfb table (HBM).
  Edge projection only for this core's 1/8 edge shard (kept in SBUF).
- pass A (per edge-half): per-incidence gather of fused rows
  (single_packet 1024-idx ops), one segment-reduce per run group yields
  U|D fused, one dma_scatter_add per chunk into UD_h [EH+128, 256] bf16.
- collectives: ReduceScatter per half (overlaps pass A of the other
  half), EA' = (U/D + ef)/D on the 1/8 shard, AllGather into EAp bf16.
- pass B: per-incidence gather of EAp rows, segment-reduce per node
  run, scatter-add into ytmp bf16 (zero-init).
- final: y = w * ytmp + bias streamed over the node table (f32 out).
"""
import os
import numpy as np
from dataclasses import dataclass

import concourse.bass as bass
import concourse.mybir as mybir
import concourse.bacc as bacc
import concourse.tile as tile
import bass_rust
from concourse.library_config import mlp as mlp_library
from concourse._compat import get_trn_type, cdiv

F32 = mybir.dt.float32
BF16 = mybir.dt.bfloat16
I16 = mybir.dt.int16
AX = mybir.AxisListType
ALU = mybir.AluOpType
ACTF = mybir.ActivationFunctionType


@dataclass
class Cfg:
    N: int = 100000
    E: int = 25000
    D: int = 128      # in dim
    H: int = 4
    C: int = 32
    NC: int = 8
    TILE_COLS: int = 32
    GOP: int = 1024   # idxs per gather op (64 descs/engine, single-packet)
    NSTAGE: int = 16  # batches per scatter chunk

    @property
    def NSH(self):
        return self.N // self.NC

    @property
    def NT(self):     # node table rows (incl dummy row)
        return cdiv(self.NSH + 1, 128) * 128

    @property
    def EH(self):     # edge-half size
        return cdiv(self.E, 256) * 128

    @property
    def SH(self):     # RS shard rows per half
        return self.EH // self.NC

    @property
    def BH(self):     # ef blocks per half
        return cdiv(self.SH, 128)

    @property
    def DUMMY_NODE(self):
        return self.NSH

    @property
    def JUNK_EDGE(self):
        return self.E


def _runs(keys):
    if len(keys) == 0:
        return (np.zeros(0, np.int64),) * 3
    change = np.flatnonzero(np.diff(keys)) + 1
    starts = np.concatenate([[0], change]).astype(np.int64)
    ends = np.concatenate([change, [len(keys)]]).astype(np.int64)
    return starts, ends - starts, keys[starts].astype(np.int64)


@dataclass
class Sched:
    batches: list          # [(k, tile, c0)]
    groups: list           # [(tile, c0, k, B, b0, chunk)]
    ntiles: int
    nchunks: int
    nbatch: int
    chunk_sizes: list


def _mk_schedule(lens_list, cfg: Cfg) -> Sched:
    # sorted-ceiling batching, common widths across cores (SPMD program)
    sorted_lens = [np.sort(np.asarray(l))[::-1] for l in lens_list]
    nbatch_total = max(cdiv(len(l), 128) for l in sorted_lens)
    batches = []
    for b in range(nbatch_total):
        w = 1
        for ls in sorted_lens:
            if b * 128 < len(ls):
                w = max(w, int(ls[b * 128]))
        batches.append(w)
    assert max(batches) <= cfg.TILE_COLS, \
        f"run length {max(batches)} > TILE_COLS"
    placed = []
    t, c = 0, 0
    for k in batches:
        if c + k > cfg.TILE_COLS:
            t += 1
            c = 0
        placed.append((k, t, c))
        c += k
    ntiles = t + 1 if placed else 1
    nbatch = len(placed)
    nchunks = cdiv(nbatch, cfg.NSTAGE)
    chunk_sizes = [min(cfg.NSTAGE, nbatch - i * cfg.NSTAGE)
                   for i in range(nchunks)]
    groups = []
    for bi, (k, t, c0) in enumerate(placed):
        ch = bi // cfg.NSTAGE
        if groups and groups[-1][0] == t and groups[-1][2] == k \
                and groups[-1][5] == ch \
                and groups[-1][1] + groups[-1][2] * groups[-1][3] == c0 \
                and groups[-1][4] + groups[-1][3] == bi:
            t0, c0g, kg, B, b0, chg = groups[-1]
            groups[-1] = (t0, c0g, kg, B + 1, b0, chg)
        else:
            groups.append((t, c0, k, 1, bi, ch))
    return Sched(placed, groups, ntiles, nchunks, nbatch, chunk_sizes)


def _wrap16(flat):
    assert len(flat) % 16 == 0
    b = flat.reshape(-1, 16).T.astype(np.int16)
    return np.tile(b, (8, 1))


def _mk_streams(sched: Sched, starts, lens, gvals, svals,
                dummy_g, junk_s, cfg: Cfg):
    """Gather stream (per 1024-idx op) + scatter stream (per chunk)."""
    TC = cfg.TILE_COLS
    g_arr = np.full((sched.ntiles, TC, 128), dummy_g, np.int64)
    s_arr = np.full((sched.nbatch, 128), junk_s, np.int64)
    order = np.argsort(-lens, kind="stable") if len(lens) else np.zeros(0, np.int64)
    for bi, (k, t, c0) in enumerate(sched.batches):
        idxs = order[bi * 128:(bi + 1) * 128]
        nr = len(idxs)
        if nr:
            st = starts[idxs]
            kr = lens[idxs]
            assert kr[0] <= k
            for kk in np.unique(kr):
                sel = np.flatnonzero(kr == kk)
                gm = gvals[st[sel][None, :] + np.arange(kk)[:, None]]
                g_arr[t, c0:c0 + kk, sel] = gm.T
            s_arr[bi, :nr] = svals[idxs]
    nops_per_tile = TC // 8
    g_idx = np.concatenate(
        [_wrap16(g_arr[t, o * 8:(o + 1) * 8].reshape(-1))
         for t in range(sched.ntiles) for o in range(nops_per_tile)], axis=1)
    sc_blocks = []
    off = 0
    for nb in sched.chunk_sizes:
        sc_blocks.append(_wrap16(s_arr[off:off + nb].reshape(-1)))
        off += nb
    sc_idx = np.concatenate(sc_blocks, axis=1)
    return g_idx, sc_idx


def build_plan(node_idx, edge_idx, cfg: Cfg):
    node_idx = np.asarray(node_idx).astype(np.int64)
    edge_idx = np.asarray(edge_idx).astype(np.int64)
    percore = []
    for m in range(cfg.NC):
        sel = np.flatnonzero(node_idx // cfg.NSH == m)
        nl = node_idx[sel] - m * cfg.NSH
        eg = edge_idx[sel]
        halves = []
        for h in (0, 1):
            msk = (eg >= h * cfg.EH) & (eg < (h + 1) * cfg.EH)
            nlh = nl[msk]
            egh = eg[msk] - h * cfg.EH
            sA, lA, vA = _runs(egh)
            halves.append(dict(nl=nlh, sA=sA, lA=lA, vA=vA))
        oB = np.argsort(nl, kind="stable")
        nB = nl[oB]
        eB = eg[oB]
        sB, lB, vB = _runs(nB)
        percore.append(dict(halves=halves, eB=eB, sB=sB, lB=lB, vB=vB))
    schedA = [_mk_schedule([c["halves"][h]["lA"] for c in percore], cfg)
              for h in (0, 1)]
    schedB = _mk_schedule([c["lB"] for c in percore], cfg)
    streams = []
    for c in percore:
        st = {}
        for h in (0, 1):
            hh = c["halves"][h]
            gA, scA = _mk_streams(
                schedA[h], hh["sA"], hh["lA"],
                gvals=hh["nl"], svals=hh["vA"],
                dummy_g=cfg.DUMMY_NODE, junk_s=cfg.EH, cfg=cfg)
            st[f"gA{h}"] = gA
            st[f"scA{h}"] = scA
        gB, scB = _mk_streams(
            schedB, c["sB"], c["lB"],
            gvals=c["eB"], svals=c["vB"],
            dummy_g=cfg.JUNK_EDGE, junk_s=cfg.DUMMY_NODE, cfg=cfg)
        st["gB"] = gB
        st["scB"] = scB
        streams.append(st)
    return schedA, schedB, streams


def _ap(t_ap, off, dims):
    base = t_ap
    part = base.ap[0]
    return bass_rust.AP(base.tensor, base.offset + off, [part] + dims)


def build_bass(cfg: Cfg, schedA, schedB, replica_groups):
    SP = os.environ.get("GNN_SP", "1") == "1"
    TC, NS, GOP = cfg.TILE_COLS, cfg.NSTAGE, cfg.GOP
    H, C = cfg.H, cfg.C
    NT, EH, SH, BH = cfg.NT, cfg.EH, cfg.SH, cfg.BH
    NSH = cfg.NSH
    ET2 = 2 * EH
    n_a_node = NT // 128
    CHK = 16
    n_node_chunks = cdiv(n_a_node, CHK)
    ops_per_tile = TC // 8

    nc = bacc.Bacc(get_trn_type() or "TRN2", target_bir_lowering=False,
                   debug=False, num_swdge_queues=4)
    _qrr = [0]

    def _q():
        _qrr[0] += 1
        return _qrr[0] % 4

    # ---- I/O ----
    xTb = nc.dram_tensor("xTb", [128, NT], BF16, kind="ExternalInput")
    haTb = nc.dram_tensor("haTb", [128, 2 * BH * 128], BF16, kind="ExternalInput")
    Wn = nc.dram_tensor("Wn", [128, H * C], BF16, kind="ExternalInput")
    We = nc.dram_tensor("We", [128, H * C], BF16, kind="ExternalInput")
    attn = nc.dram_tensor("attn", [128, H * C], BF16, kind="ExternalInput")
    bias_t = nc.dram_tensor("bias_t", [128, H * C], F32, kind="ExternalInput")
    gA_i = [nc.dram_tensor(f"gA{h}_i", [128, schedA[h].ntiles * ops_per_tile * (GOP // 16)],
                           I16, kind="ExternalInput") for h in (0, 1)]
    scA_i = [nc.dram_tensor(f"scA{h}_i", [128, schedA[h].nbatch * 8], I16,
                            kind="ExternalInput") for h in (0, 1)]
    gB_i = nc.dram_tensor("gB_i", [128, schedB.ntiles * ops_per_tile * (GOP // 16)],
                          I16, kind="ExternalInput")
    scB_i = nc.dram_tensor("scB_i", [128, schedB.nbatch * 8], I16,
                           kind="ExternalInput")
    y = nc.dram_tensor("y", [NT, H * C], F32, kind="ExternalOutput")

    # ---- internal DRAM ----
    nfb = nc.dram_tensor("nfb", [NT, 256], BF16)       # [w*nf | w*4 | pad]
    UD = [nc.dram_tensor(f"UD{h}", [EH + 128, 256], BF16) for h in (0, 1)]
    UDr = [nc.dram_tensor(f"UDr{h}", [SH, 256], BF16) for h in (0, 1)]
    EAsh = [nc.dram_tensor(f"EAsh{h}", [SH, 128], BF16) for h in (0, 1)]
    EAp = nc.dram_tensor("EAp", [ET2, 128], BF16, addr_space="Shared")
    ytmp = nc.dram_tensor("ytmp", [NT, 128], BF16)

    nfb_v = nfb[:].rearrange("(a p) c -> p a c", p=128)
    ud_v = [UD[h][:].rearrange("(a p) c -> p a c", p=128) for h in (0, 1)]
    ytmp_v = ytmp[:].rearrange("(a p) c -> p a c", p=128)
    y_v = y[:].rearrange("(a p) c -> p a c", p=128)

    with tile.TileContext(nc) as tc, \
            nc.allow_low_precision(reason="2e-2 tolerance, bf16 throughout"), \
            tc.tile_pool(name="const", bufs=1) as cpool:
        if True:
            nc.gpsimd.load_library(mlp_library)
            Wn_sb = cpool.tile([128, H * C], BF16)
            We_sb = cpool.tile([128, H * C], BF16)
            attn_sb = cpool.tile([128, H * C], BF16)
            bias_sb = cpool.tile([128, H * C], F32)
            zeros_sb = cpool.tile([128, 4096], BF16)
            w_all = cpool.tile([128, n_a_node, H], BF16)
            ef_sb = cpool.tile([128, 2 * BH, 128], BF16)
            nc.sync.dma_start(Wn_sb[:], Wn[:])
            nc.sync.dma_start(We_sb[:], We[:])
            nc.sync.dma_start(attn_sb[:], attn[:])
            nc.sync.dma_start(bias_sb[:], bias_t[:])
            nc.gpsimd.memset(zeros_sb[:], 0.0)

            # ---- init: UD=0, ytmp=0 ----
            zv16 = zeros_sb[:].rearrange("p (a c) -> p a c", c=256)  # [128,16,256]
            for h in (0, 1):
                na = (EH + 128) // 128
                for a0 in range(0, na, 16):
                    aa = min(16, na - a0)
                    nc.sync.dma_start(ud_v[h][:, a0:a0 + aa, :], zv16[:, :aa, :])
            zv8 = zeros_sb[:].rearrange("p (a c) -> p a c", c=128)   # [128,32,128]
            for a0 in range(0, n_a_node, 32):
                aa = min(32, n_a_node - a0)
                nc.sync.dma_start(ytmp_v[:, a0:a0 + aa, :], zv8[:, :aa, :])

            # ---- phase 1: projections ----
            with (tc.tile_pool(name="p1", bufs=2) as p1,
                  tc.tile_pool(name="p1s", bufs=2) as p1s,
                  tc.tile_pool(name="ps", bufs=4, space="PSUM") as psp):
                # node side: fused pre-weighted rows
                for ci in range(n_node_chunks):
                    a0 = ci * CHK
                    aa = min(CHK, n_a_node - a0)
                    xc = p1.tile([128, CHK * 128], BF16, tag="xc")
                    nc.sync.dma_start(xc[:, :aa * 128], xTb[:, a0 * 128:(a0 + aa) * 128])
                    nfst = p1s.tile([128, CHK, 256], BF16, tag="nfst")
                    for i in range(aa):
                        mm = psp.tile([128, 128], F32, tag="mm")
                        nc.tensor.matmul(mm[:], xc[:, i * 128:(i + 1) * 128],
                                         Wn_sb[:], start=True, stop=True)
                        nc.vector.tensor_copy(nfst[:, i, 0:128], mm[:])
                    # attn dot -> praw -> w = exp(praw)
                    tmp1 = p1s.tile([128, CHK, 128], BF16, tag="tmp1")
                    nc.vector.tensor_tensor(
                        out=tmp1[:, :aa, :].rearrange("p a (h c) -> p a h c", h=H),
                        in0=nfst[:, :aa, 0:128].rearrange("p a (h c) -> p a h c", h=H),
                        in1=attn_sb[:].rearrange("p (h c) -> p h c", h=H)
                            .unsqueeze(1).broadcast_to([128, aa, H, C]),
                        op=ALU.mult)
                    praw = p1s.tile([128, CHK, H], F32, tag="praw")
                    nc.vector.tensor_reduce(
                        out=praw[:, :aa, :],
                        in_=tmp1[:, :aa, :].rearrange("p a (h c) -> p a h c", h=H),
                        axis=AX.X, op=ALU.add)
                    west = p1s.tile([128, CHK, H], BF16, tag="west")
                    nc.scalar.activation(west[:, :aa, :], praw[:, :aa, :], ACTF.Exp)
                    nc.vector.tensor_copy(w_all[:, a0:a0 + aa, :], west[:, :aa, :])
                    # fused row: [w*nf | w | pad124]
                    nc.vector.tensor_tensor(
                        out=_ap(nfst[:], 0, [[256, aa], [32, H], [1, C]]),
                        in0=_ap(nfst[:], 0, [[256, aa], [32, H], [1, C]]),
                        in1=_ap(west[:], 0, [[H, aa], [1, H], [0, C]]),
                        op=ALU.mult)
                    nc.vector.tensor_copy(nfst[:, :aa, 128:128 + H], west[:, :aa, :])
                    nc.gpsimd.memset(
                        _ap(nfst[:], 128 + H, [[256, aa], [1, 128 - H]]), 0.0)
                    nc.sync.dma_start(nfb_v[:, a0:a0 + aa, :], nfst[:, :aa, :])
                # zero dummy node rows (NSH..NT): all-zero fused rows
                nc.sync.dma_start(nfb[NSH:NT, :], zeros_sb[0:NT - NSH, 0:256])
                # edge shard projection (both halves)
                hc = p1.tile([128, 2 * BH * 128], BF16, tag="hc")
                nc.sync.dma_start(hc[:], haTb[:])
                for b in range(2 * BH):
                    mm = psp.tile([128, 128], F32, tag="mm")
                    nc.tensor.matmul(mm[:], hc[:, b * 128:(b + 1) * 128],
                                     We_sb[:], start=True, stop=True)
                    nc.vector.tensor_copy(ef_sb[:, b, :], mm[:])

            # ---- pass A (both halves) + per-half ReduceScatter ----
            with (tc.tile_pool(name="gA", bufs=2) as gpool,
                  tc.tile_pool(name="stA", bufs=2) as spool,
                  tc.tile_pool(name="idxA", bufs=1) as ipool):
                gA_sb = [ipool.tile([128, gA_i[h].shape[1]], I16,
                                    name=f"gA{h}_sb") for h in (0, 1)]
                scA_sb = [ipool.tile([128, scA_i[h].shape[1]], I16,
                                     name=f"scA{h}_sb") for h in (0, 1)]
                gB_sb = ipool.tile([128, gB_i.shape[1]], I16)
                scB_sb = ipool.tile([128, scB_i.shape[1]], I16)
                for h in (0, 1):
                    nc.sync.dma_start(gA_sb[h][:], gA_i[h][:])
                    nc.sync.dma_start(scA_sb[h][:], scA_i[h][:])
                nc.sync.dma_start(gB_sb[:], gB_i[:])
                nc.sync.dma_start(scB_sb[:], scB_i[:])

                def run_pass(sched, g_sb, sc_sb, src_ap, dst, elem, width,
                             tag):
                    groups_by_tile = {}
                    for g in sched.groups:
                        groups_by_tile.setdefault(g[0], []).append(g)
                    state = {"chunk": 0, "sc_off": 0, "stag": None}

                    def open_chunk():
                        state["stag"] = spool.tile([128, NS, elem], BF16,
                                                   tag=f"stag{tag}",
                                                   name=f"stag{tag}")

                    def flush_chunk():
                        ch = state["chunk"]
                        nb = sched.chunk_sizes[ch]
                        nc.gpsimd.dma_scatter_add(
                            dst[:, 0:elem], state["stag"][:, :nb, :],
                            sc_sb[:, state["sc_off"]:state["sc_off"] + nb * 8],
                            nb * 128, nb * 128, elem, elem_step=width,
                            single_packet=False, queue_num=_q())
                        state["sc_off"] += nb * 8
                        state["chunk"] += 1

                    open_chunk()
                    for t in range(sched.ntiles):
                        G = gpool.tile([128, TC, width], BF16, tag=f"G{tag}")
                        for o in range(ops_per_tile):
                            nc.gpsimd.dma_gather(
                                G[:, o * 8:(o + 1) * 8, :], src_ap,
                                g_sb[:, (t * ops_per_tile + o) * (GOP // 16):
                                     (t * ops_per_tile + o + 1) * (GOP // 16)],
                                GOP, GOP, width, single_packet=SP,
                                queue_num=_q())
                        for (_, c0, k, B, b0, ch) in groups_by_tile.get(t, []):
                            if ch != state["chunk"]:
                                flush_chunk()
                                open_chunk()
                            bpos = b0 - ch * NS
                            nc.vector.tensor_reduce(
                                out=state["stag"][:, bpos:bpos + B, :],
                                in_=_ap(G[:], c0 * width,
                                        [[k * width, B], [1, elem], [width, k]]),
                                axis=AX.X, op=ALU.add)
                    flush_chunk()

                for h in (0, 1):
                    run_pass(schedA[h], gA_sb[h], scA_sb[h], nfb[:], UD[h],
                             elem=128 + H, width=256, tag=f"A{h}")
                    nc.gpsimd.collective_compute(
                        "ReduceScatter", ALU.add, replica_groups=replica_groups,
                        ins=[UD[h][0:EH, :]], outs=[UDr[h][:]])

                # ---- EA' = (U/D + ef)/D on shard, then AllGather ----
                with tc.tile_pool(name="ea", bufs=3) as eap:
                    for h in (0, 1):
                        for i in range(BH):
                            r0 = i * 128
                            P = min(128, SH - r0)
                            dchunk = eap.tile([128, 256], BF16, tag="dchunk")
                            nc.sync.dma_start(dchunk[0:P, :], UDr[h][r0:r0 + P, :])
                            d4 = eap.tile([128, H], F32, tag="d4")
                            nc.vector.tensor_copy(d4[0:P, :], dchunk[0:P, 128:128 + H])
                            nc.vector.tensor_scalar_add(d4[0:P, :], d4[0:P, :], 1e-20)
                            inv = eap.tile([128, H], BF16, tag="inv")
                            nc.vector.reciprocal(d4[0:P, :], d4[0:P, :])
                            nc.vector.tensor_copy(inv[0:P, :], d4[0:P, :])
                            uview = _ap(dchunk[0:P, :], 0, [[32, H], [1, C]])
                            iview = _ap(inv[0:P, :], 0, [[1, H], [0, C]])
                            nc.vector.tensor_tensor(out=uview, in0=uview,
                                                    in1=iview, op=ALU.mult)
                            nc.vector.tensor_tensor(
                                out=dchunk[0:P, 0:128], in0=dchunk[0:P, 0:128],
                                in1=ef_sb[0:P, h * BH + i, :], op=ALU.add)
                            nc.vector.tensor_tensor(out=uview, in0=uview,
                                                    in1=iview, op=ALU.mult)
                            nc.sync.dma_start(EAsh[h][r0:r0 + P, :],
                                              dchunk[0:P, 0:128])
                        nc.gpsimd.collective_compute(
                            "AllGather", ALU.bypass, replica_groups=replica_groups,
                            ins=[EAsh[h][:]], outs=[EAp[h * EH:(h + 1) * EH, :]])
                    # zero junk edge rows [E, ET2)
                    nj = ET2 - cfg.E
                    nc.sync.dma_start(EAp[cfg.E:ET2, :], zeros_sb[0:nj, 0:128])

                # ---- pass B ----
                run_pass(schedB, gB_sb, scB_sb, EAp[:], ytmp,
                         elem=128, width=128, tag="B")

            # ---- final: y = w * ytmp + bias ----
            with tc.tile_pool(name="fin", bufs=2) as fpool:
                for ci in range(n_node_chunks):
                    a0 = ci * CHK
                    aa = min(CHK, n_a_node - a0)
                    ytc = fpool.tile([128, CHK, 128], BF16, tag="ytc")
                    nc.sync.dma_start(ytc[:, :aa, :], ytmp_v[:, a0:a0 + aa, :])
                    yst = fpool.tile([128, CHK, 128], F32, tag="yst")
                    nc.vector.tensor_tensor(
                        out=_ap(yst[:], 0, [[128, aa], [32, H], [1, C]]),
                        in0=_ap(ytc[:], 0, [[128, aa], [32, H], [1, C]]),
                        in1=_ap(w_all[:], a0 * H, [[H, aa], [1, H], [0, C]]),
                        op=ALU.mult)
                    nc.vector.tensor_tensor(
                        out=yst[:, :aa, :], in0=yst[:, :aa, :],
                        in1=bias_sb[:].unsqueeze(1).broadcast_to([128, aa, 128]),
                        op=ALU.add)
                    nc.sync.dma_start(y_v[:, a0:a0 + aa, :], yst[:, :aa, :])
    nc.compile()
    return nc


def host_inputs(cfg: Cfg, x, ha, W_node, W_edge, attn_l, bias, streams):
    import ml_dtypes
    bf16 = ml_dtypes.bfloat16
    x = np.asarray(x, np.float32)
    ha = np.asarray(ha, np.float32)
    W_node = np.asarray(W_node, np.float32)
    W_edge = np.asarray(W_edge, np.float32)
    attn_flat = np.asarray(attn_l, np.float32).reshape(-1)
    bias = np.asarray(bias, np.float32).reshape(-1)
    attn_rep = np.tile(attn_flat[None, :], (128, 1)).astype(bf16)
    bias_rep = np.tile(bias[None, :], (128, 1)).astype(np.float32)
    ha_pad = np.zeros((2 * cfg.EH, cfg.D), np.float32)
    ha_pad[:cfg.E] = ha
    in_maps = []
    for m in range(cfg.NC):
        xs = np.zeros((cfg.NT, cfg.D), np.float32)
        xs[:cfg.NSH] = x[m * cfg.NSH:(m + 1) * cfg.NSH]
        # edge shard columns: per half, BH blocks of 128 starting at
        # h*EH + m*SH (rows beyond the shard zeroed)
        haT_shard = np.zeros((cfg.D, 2 * cfg.BH * 128), np.float32)
        for h in (0, 1):
            base = h * cfg.EH + m * cfg.SH
            blk = ha_pad[base:base + cfg.SH].T    # [D, SH]
            haT_shard[:, h * cfg.BH * 128:h * cfg.BH * 128 + cfg.SH] = blk
        st = streams[m]
        in_maps.append({
            "xTb": np.ascontiguousarray(xs.T).astype(bf16),
            "haTb": haT_shard.astype(bf16),
            "Wn": W_node.astype(bf16), "We": W_edge.astype(bf16),
            "attn": attn_rep, "bias_t": bias_rep,
            "gA0_i": st["gA0"], "scA0_i": st["scA0"],
            "gA1_i": st["gA1"], "scA1_i": st["scA1"],
            "gB_i": st["gB"], "scB_i": st["scB"],
        })
    return in_maps


# ======================== public entry point ========================
_CFG = Cfg()
LAST_RESULTS = None


def _install_axon_ntff_shim():
    import sys, types, ctypes, contextlib
    import concourse.bass_utils as bu
    bu.upload_artifacts = lambda d: str(d)
    try:
        from antenv.axon_hooks import get_axon_ntff_profile_hook  # noqa
        return
    except ImportError:
        pass
    so_path = "/opt/axon/libaxon_pjrt.so"
    try:
        lib = ctypes.CDLL(so_path)
    except OSError:
        return
    if not hasattr(lib, "axon_start_nrt_profile"):
        return
    lib.axon_start_nrt_profile.argtypes = [ctypes.POINTER(ctypes.c_int64),
                                           ctypes.c_size_t]
    lib.axon_start_nrt_profile.restype = ctypes.c_int64
    lib.axon_stop_nrt_profile.argtypes = [ctypes.c_char_p]
    lib.axon_stop_nrt_profile.restype = ctypes.c_int64

    @contextlib.contextmanager
    def _hook(output_dir, device_ids):
        import jax
        jax.devices()
        if device_ids:
            ids = (ctypes.c_int64 * len(device_ids))(*device_ids)
            rc = lib.axon_start_nrt_profile(ids, len(device_ids))
        else:
            rc = lib.axon_start_nrt_profile(None, 0)
        if rc != 0:
            raise RuntimeError(f"axon_start_nrt_profile rc={rc}")
        try:
            yield
        finally:
            n = lib.axon_stop_nrt_profile(str(output_dir).encode())
            print(f"ntff profile: {n} file(s) -> {output_dir}")

    mod = types.ModuleType("antenv.axon_hooks")
    mod.get_axon_ntff_profile_hook = lambda: _hook
    mod.set_axon_ntff_profile_hook = lambda h: None
    sys.modules["antenv.axon_hooks"] = mod


def kernel(**inputs) -> np.ndarray:
    from concourse.bass_utils import run_bass_kernel_spmd
    cfg = _CFG
    x = np.asarray(inputs["x"], np.float32)
    ha = np.asarray(inputs["hyperedge_attr"], np.float32)
    node_idx = np.asarray(inputs["node_idx"]).astype(np.int64)
    edge_idx = np.asarray(inputs["edge_idx"]).astype(np.int64)
    schedA, schedB, streams = build_plan(node_idx, edge_idx, cfg)
    nc = build_bass(cfg, schedA, schedB, [list(range(cfg.NC))])
    in_maps = host_inputs(cfg, x, ha, inputs["W_node"], inputs["W_edge"],
                          inputs["attn_l"], inputs["bias"], streams)
    trace = os.environ.get("GNN_TRACE", "0") == "1"
    if trace:
        _install_axon_ntff_shim()
    res = run_bass_kernel_spmd(nc, in_maps, list(range(cfg.NC)), trace=trace)
    global LAST_RESULTS
    LAST_RESULTS = res
    out = np.concatenate(
        [np.asarray(res.results[m]["y"], dtype=np.float32)[:cfg.NSH]
         for m in range(cfg.NC)], axis=0)
    return np.ascontiguousarray(out, dtype=np.float32)


# revision 18
# speedup vs baseline: 1.4599x; 1.0594x over previous
"""Hypergraph conv kernel for TRN2 (8 NeuronCores), v2.

Design (per core, incidences sharded by node id):
- phase 1: node projection nf = x@Wn, w = exp((nf*attn).sum per head);
  fused bf16 gather rows [w*nf (128ch) | w (4) | pad] -> nfb table (HBM).
  Edge projection only for this core's 1/8 edge shard (kept in SBUF).
- pass A (per edge-half): per-incidence gather of fused rows
  (single_packet 1024-idx ops), one segment-reduce per run group yields
  U|D fused, one dma_scatter_add per chunk into UD_h [EH+128, 256] bf16.
- collectives: ReduceScatter per half (overlaps pass A of the other
  half), EA' = (U/D + ef)/D on the 1/8 shard, AllGather into EAp bf16.
- pass B: per-incidence gather of EAp rows, segment-reduce per node
  run, scatter-add into ytmp bf16 (zero-init).
- final: y = w * ytmp + bias streamed over the node table (f32 out).
"""
import os
import numpy as np
from dataclasses import dataclass

import concourse.bass as bass
import concourse.mybir as mybir
import concourse.bacc as bacc
import concourse.tile as tile
import bass_rust
from concourse.library_config import mlp as mlp_library
from concourse._compat import get_trn_type, cdiv

F32 = mybir.dt.float32
BF16 = mybir.dt.bfloat16
I16 = mybir.dt.int16
AX = mybir.AxisListType
ALU = mybir.AluOpType
ACTF = mybir.ActivationFunctionType


@dataclass
class Cfg:
    N: int = 100000
    E: int = 25000
    D: int = 128      # in dim
    H: int = 4
    C: int = 32
    NC: int = 8
    TILE_COLS: int = 32
    GOP: int = 1024   # idxs per gather op (64 descs/engine, single-packet)
    NSTAGE: int = 16  # batches per scatter chunk

    @property
    def NSH(self):
        return self.N // self.NC

    @property
    def NT(self):     # node table rows (incl dummy row)
        return cdiv(self.NSH + 1, 128) * 128

    @property
    def ET2(self):    # edge table rows (excl junk)
        return cdiv(self.E, 256) * 256

    @property
    def SH(self):     # RS shard rows
        return self.ET2 // self.NC

    @property
    def BH(self):     # ef blocks
        return cdiv(self.SH, 128)

    @property
    def DUMMY_NODE(self):
        return self.NSH

    @property
    def JUNK_EDGE(self):
        return self.E


def _runs(keys):
    if len(keys) == 0:
        return (np.zeros(0, np.int64),) * 3
    change = np.flatnonzero(np.diff(keys)) + 1
    starts = np.concatenate([[0], change]).astype(np.int64)
    ends = np.concatenate([change, [len(keys)]]).astype(np.int64)
    return starts, ends - starts, keys[starts].astype(np.int64)


@dataclass
class Sched:
    batches: list          # [(k, tile, c0)]
    groups: list           # [(tile, c0, k, B, b0, chunk)]
    ntiles: int
    nchunks: int
    nbatch: int
    chunk_sizes: list


def _mk_schedule(lens_list, cfg: Cfg) -> Sched:
    # sorted-ceiling batching, common widths across cores (SPMD program)
    sorted_lens = [np.sort(np.asarray(l))[::-1] for l in lens_list]
    nbatch_total = max(cdiv(len(l), 128) for l in sorted_lens)
    batches = []
    for b in range(nbatch_total):
        w = 1
        for ls in sorted_lens:
            if b * 128 < len(ls):
                w = max(w, int(ls[b * 128]))
        batches.append(w)
    assert max(batches) <= cfg.TILE_COLS, \
        f"run length {max(batches)} > TILE_COLS"
    placed = []
    t, c = 0, 0
    for k in batches:
        if c + k > cfg.TILE_COLS:
            t += 1
            c = 0
        placed.append((k, t, c))
        c += k
    ntiles = t + 1 if placed else 1
    nbatch = len(placed)
    nchunks = cdiv(nbatch, cfg.NSTAGE)
    chunk_sizes = [min(cfg.NSTAGE, nbatch - i * cfg.NSTAGE)
                   for i in range(nchunks)]
    groups = []
    for bi, (k, t, c0) in enumerate(placed):
        ch = bi // cfg.NSTAGE
        if groups and groups[-1][0] == t and groups[-1][2] == k \
                and groups[-1][5] == ch \
                and groups[-1][1] + groups[-1][2] * groups[-1][3] == c0 \
                and groups[-1][4] + groups[-1][3] == bi:
            t0, c0g, kg, B, b0, chg = groups[-1]
            groups[-1] = (t0, c0g, kg, B + 1, b0, chg)
        else:
            groups.append((t, c0, k, 1, bi, ch))
    return Sched(placed, groups, ntiles, nchunks, nbatch, chunk_sizes)


def _wrap16(flat):
    assert len(flat) % 16 == 0
    b = flat.reshape(-1, 16).T.astype(np.int16)
    return np.tile(b, (8, 1))


def _mk_streams(sched: Sched, starts, lens, gvals, svals,
                dummy_g, junk_s, cfg: Cfg):
    """Gather stream (per 1024-idx op) + scatter stream (per chunk)."""
    TC = cfg.TILE_COLS
    g_arr = np.full((sched.ntiles, TC, 128), dummy_g, np.int64)
    s_arr = np.full((sched.nbatch, 128), junk_s, np.int64)
    order = np.argsort(-lens, kind="stable") if len(lens) else np.zeros(0, np.int64)
    for bi, (k, t, c0) in enumerate(sched.batches):
        idxs = order[bi * 128:(bi + 1) * 128]
        nr = len(idxs)
        if nr:
            st = starts[idxs]
            kr = lens[idxs]
            assert kr[0] <= k
            for kk in np.unique(kr):
                sel = np.flatnonzero(kr == kk)
                gm = gvals[st[sel][None, :] + np.arange(kk)[:, None]]
                g_arr[t, c0:c0 + kk, sel] = gm.T
            s_arr[bi, :nr] = svals[idxs]
    nops_per_tile = TC // 8
    g_idx = np.concatenate(
        [_wrap16(g_arr[t, o * 8:(o + 1) * 8].reshape(-1))
         for t in range(sched.ntiles) for o in range(nops_per_tile)], axis=1)
    sc_blocks = []
    off = 0
    for nb in sched.chunk_sizes:
        sc_blocks.append(_wrap16(s_arr[off:off + nb].reshape(-1)))
        off += nb
    sc_idx = np.concatenate(sc_blocks, axis=1)
    return g_idx, sc_idx


def build_plan(node_idx, edge_idx, cfg: Cfg):
    node_idx = np.asarray(node_idx).astype(np.int64)
    edge_idx = np.asarray(edge_idx).astype(np.int64)
    percore = []
    for m in range(cfg.NC):
        sel = np.flatnonzero(node_idx // cfg.NSH == m)
        nl = node_idx[sel] - m * cfg.NSH
        eg = edge_idx[sel]
        sA, lA, vA = _runs(eg)
        oB = np.argsort(nl, kind="stable")
        nB = nl[oB]
        eB = eg[oB]
        sB, lB, vB = _runs(nB)
        percore.append(dict(nl=nl, sA=sA, lA=lA, vA=vA,
                            eB=eB, sB=sB, lB=lB, vB=vB))
    schedA = _mk_schedule([c["lA"] for c in percore], cfg)
    schedB = _mk_schedule([c["lB"] for c in percore], cfg)
    streams = []
    for c in percore:
        st = {}
        gA, scA = _mk_streams(
            schedA, c["sA"], c["lA"],
            gvals=c["nl"], svals=c["vA"],
            dummy_g=cfg.DUMMY_NODE, junk_s=cfg.ET2, cfg=cfg)
        st["gA"] = gA
        st["scA"] = scA
        gB, scB = _mk_streams(
            schedB, c["sB"], c["lB"],
            gvals=c["eB"], svals=c["vB"],
            dummy_g=cfg.JUNK_EDGE, junk_s=cfg.DUMMY_NODE, cfg=cfg)
        st["gB"] = gB
        st["scB"] = scB
        streams.append(st)
    return schedA, schedB, streams


def _ap(t_ap, off, dims):
    base = t_ap
    part = base.ap[0]
    return bass_rust.AP(base.tensor, base.offset + off, [part] + dims)


def _collective_on(nc, eng, kind, op, replica_groups, in_ap, out_ap):
    """collective_compute issued from an arbitrary engine so it does not
    block the Pool (gpsimd) queue. Mirrors BassGpSimd.collective_compute
    for the internal-DRAM cases used here."""
    from concourse.bass import filter_and_check_groups
    nc.has_collectives = True
    rg = filter_and_check_groups(nc.num_devices, replica_groups)
    return eng.add_instruction(
        mybir.InstCollectiveCompute(
            name=f"I-{nc.next_id()}",
            kind=kind,
            op=op,
            replica_groups=rg,
            ins=[eng.lower_ap(in_ap)],
            outs=[eng.lower_ap(out_ap)],
            unique_tensors="No",
            cc_dim="Partition",
        )
    )


def _fold_reduce(nc, G, stag, c0, k, B, bpos, width, elem):
    """Segment sum over k columns per run via fold-tree tensor_tensor adds
    (innermost stride 1 -> DVE 2x perf mode), final fold/copy into stag."""
    W = width
    kk = k
    while kk > 2:
        h = kk // 2
        rem = kk - h
        nc.vector.tensor_tensor(
            out=_ap(G[:], c0 * W, [[k * W, B], [W, h], [1, elem]]),
            in0=_ap(G[:], c0 * W, [[k * W, B], [W, h], [1, elem]]),
            in1=_ap(G[:], (c0 + rem) * W, [[k * W, B], [W, h], [1, elem]]),
            op=ALU.add)
        kk = rem
    if kk == 2:
        nc.vector.tensor_tensor(
            out=_ap(stag[:], bpos * elem, [[elem, B], [elem, 1], [1, elem]]),
            in0=_ap(G[:], c0 * W, [[k * W, B], [W, 1], [1, elem]]),
            in1=_ap(G[:], (c0 + 1) * W, [[k * W, B], [W, 1], [1, elem]]),
            op=ALU.add)
    else:
        nc.vector.tensor_copy(
            _ap(stag[:], bpos * elem, [[elem, B], [1, elem]]),
            _ap(G[:], c0 * W, [[k * W, B], [1, elem]]))


def build_bass(cfg: Cfg, schedA, schedB, replica_groups):
    SP = os.environ.get("GNN_SP", "1") == "1"
    TC, NS, GOP = cfg.TILE_COLS, cfg.NSTAGE, cfg.GOP
    H, C = cfg.H, cfg.C
    NT, ET2, SH, BH = cfg.NT, cfg.ET2, cfg.SH, cfg.BH
    NSH = cfg.NSH
    n_a_node = NT // 128
    CHK = 16
    n_node_chunks = cdiv(n_a_node, CHK)
    ops_per_tile = TC // 8

    nc = bacc.Bacc(get_trn_type() or "TRN2", target_bir_lowering=False,
                   debug=False, num_swdge_queues=4)
    _qrr = [0]

    def _q():
        _qrr[0] += 1
        return _qrr[0] % 4

    # ---- I/O ----
    xTb = nc.dram_tensor("xTb", [128, NT], BF16, kind="ExternalInput")
    haTb = nc.dram_tensor("haTb", [128, BH * 128], BF16, kind="ExternalInput")
    Wn = nc.dram_tensor("Wn", [128, H * C], BF16, kind="ExternalInput")
    We = nc.dram_tensor("We", [128, H * C], BF16, kind="ExternalInput")
    attn = nc.dram_tensor("attn", [128, H * C], BF16, kind="ExternalInput")
    bias_t = nc.dram_tensor("bias_t", [128, H * C], F32, kind="ExternalInput")
    gA_i = nc.dram_tensor("gA_i", [128, schedA.ntiles * ops_per_tile * (GOP // 16)],
                          I16, kind="ExternalInput")
    scA_i = nc.dram_tensor("scA_i", [128, schedA.nbatch * 8], I16,
                           kind="ExternalInput")
    gB_i = nc.dram_tensor("gB_i", [128, schedB.ntiles * ops_per_tile * (GOP // 16)],
                          I16, kind="ExternalInput")
    scB_i = nc.dram_tensor("scB_i", [128, schedB.nbatch * 8], I16,
                           kind="ExternalInput")
    y = nc.dram_tensor("y", [NT, H * C], F32, kind="ExternalOutput")

    # ---- internal DRAM ----
    nfb = nc.dram_tensor("nfb", [NT, 256], BF16)       # [w*nf | w*4 | pad]
    UD = nc.dram_tensor("UD", [ET2 + 128, 256], BF16)
    UDr = nc.dram_tensor("UDr", [SH, 256], BF16)
    EAsh = nc.dram_tensor("EAsh", [SH, 128], BF16)
    EAp = nc.dram_tensor("EAp", [ET2, 128], BF16, addr_space="Shared")
    ytmp = nc.dram_tensor("ytmp", [NT, 128], BF16)

    nfb_v = nfb[:].rearrange("(a p) c -> p a c", p=128)
    ud_v = UD[:].rearrange("(a p) c -> p a c", p=128)
    ytmp_v = ytmp[:].rearrange("(a p) c -> p a c", p=128)
    y_v = y[:].rearrange("(a p) c -> p a c", p=128)

    with tile.TileContext(nc) as tc, \
            nc.allow_low_precision(reason="2e-2 tolerance, bf16 throughout"), \
            tc.tile_pool(name="const", bufs=1) as cpool:
        if True:
            nc.gpsimd.load_library(mlp_library)
            Wn_sb = cpool.tile([128, H * C], BF16)
            We_sb = cpool.tile([128, H * C], BF16)
            attn_sb = cpool.tile([128, H * C], BF16)
            bias_sb = cpool.tile([128, H * C], F32)
            zeros_sb = cpool.tile([128, 4096], BF16)
            w_all = cpool.tile([128, n_a_node, H], BF16)
            ef_sb = cpool.tile([128, BH, 128], BF16)
            nc.sync.dma_start(Wn_sb[:], Wn[:])
            nc.sync.dma_start(We_sb[:], We[:])
            nc.sync.dma_start(attn_sb[:], attn[:])
            nc.sync.dma_start(bias_sb[:], bias_t[:])
            nc.gpsimd.memset(zeros_sb[:], 0.0)

            # ---- init: UD=0, ytmp=0 ----
            zv16 = zeros_sb[:].rearrange("p (a c) -> p a c", c=256)  # [128,16,256]
            na = (ET2 + 128) // 128
            for a0 in range(0, na, 16):
                aa = min(16, na - a0)
                nc.sync.dma_start(ud_v[:, a0:a0 + aa, :], zv16[:, :aa, :])
            zv8 = zeros_sb[:].rearrange("p (a c) -> p a c", c=128)   # [128,32,128]
            for a0 in range(0, n_a_node, 32):
                aa = min(32, n_a_node - a0)
                nc.sync.dma_start(ytmp_v[:, a0:a0 + aa, :], zv8[:, :aa, :])

            # ---- phase 1: projections ----
            with (tc.tile_pool(name="p1", bufs=2) as p1,
                  tc.tile_pool(name="p1s", bufs=2) as p1s,
                  tc.tile_pool(name="ps", bufs=4, space="PSUM") as psp):
                # node side: fused pre-weighted rows
                for ci in range(n_node_chunks):
                    a0 = ci * CHK
                    aa = min(CHK, n_a_node - a0)
                    xc = p1.tile([128, CHK * 128], BF16, tag="xc")
                    nc.sync.dma_start(xc[:, :aa * 128], xTb[:, a0 * 128:(a0 + aa) * 128])
                    nfst = p1s.tile([128, CHK, 256], BF16, tag="nfst")
                    for i in range(aa):
                        mm = psp.tile([128, 128], F32, tag="mm")
                        nc.tensor.matmul(mm[:], xc[:, i * 128:(i + 1) * 128],
                                         Wn_sb[:], start=True, stop=True)
                        nc.vector.tensor_copy(nfst[:, i, 0:128], mm[:])
                    # attn dot -> praw -> w = exp(praw)
                    tmp1 = p1s.tile([128, CHK, 128], BF16, tag="tmp1")
                    nc.vector.tensor_tensor(
                        out=tmp1[:, :aa, :].rearrange("p a (h c) -> p a h c", h=H),
                        in0=nfst[:, :aa, 0:128].rearrange("p a (h c) -> p a h c", h=H),
                        in1=attn_sb[:].rearrange("p (h c) -> p h c", h=H)
                            .unsqueeze(1).broadcast_to([128, aa, H, C]),
                        op=ALU.mult)
                    praw = p1s.tile([128, CHK, H], F32, tag="praw")
                    nc.vector.tensor_reduce(
                        out=praw[:, :aa, :],
                        in_=tmp1[:, :aa, :].rearrange("p a (h c) -> p a h c", h=H),
                        axis=AX.X, op=ALU.add)
                    west = p1s.tile([128, CHK, H], BF16, tag="west")
                    nc.scalar.activation(west[:, :aa, :], praw[:, :aa, :], ACTF.Exp)
                    nc.vector.tensor_copy(w_all[:, a0:a0 + aa, :], west[:, :aa, :])
                    # fused row: [w*nf | w | pad124]
                    nc.vector.tensor_tensor(
                        out=_ap(nfst[:], 0, [[256, aa], [32, H], [1, C]]),
                        in0=_ap(nfst[:], 0, [[256, aa], [32, H], [1, C]]),
                        in1=_ap(west[:], 0, [[H, aa], [1, H], [0, C]]),
                        op=ALU.mult)
                    nc.vector.tensor_copy(nfst[:, :aa, 128:128 + H], west[:, :aa, :])
                    nc.gpsimd.memset(
                        _ap(nfst[:], 128 + H, [[256, aa], [1, 128 - H]]), 0.0)
                    nc.sync.dma_start(nfb_v[:, a0:a0 + aa, :], nfst[:, :aa, :])
                # zero dummy node rows (NSH..NT): all-zero fused rows
                nc.sync.dma_start(nfb[NSH:NT, :], zeros_sb[0:NT - NSH, 0:256])


            # ---- pass A (both halves) + per-half ReduceScatter ----
            with (tc.tile_pool(name="gA", bufs=2) as gpool,
                  tc.tile_pool(name="stA", bufs=2) as spool,
                  tc.tile_pool(name="idxA", bufs=1) as ipool):
                gA_sb = ipool.tile([128, gA_i.shape[1]], I16)
                scA_sb = ipool.tile([128, scA_i.shape[1]], I16)
                gB_sb = ipool.tile([128, gB_i.shape[1]], I16)
                scB_sb = ipool.tile([128, scB_i.shape[1]], I16)
                nc.sync.dma_start(gA_sb[:], gA_i[:])
                nc.sync.dma_start(scA_sb[:], scA_i[:])
                nc.sync.dma_start(gB_sb[:], gB_i[:])
                nc.sync.dma_start(scB_sb[:], scB_i[:])

                def run_pass(sched, g_sb, sc_sb, src_ap, dst, elem, width,
                             tag):
                    groups_by_tile = {}
                    for g in sched.groups:
                        groups_by_tile.setdefault(g[0], []).append(g)
                    state = {"chunk": 0, "sc_off": 0, "stag": None}

                    def open_chunk():
                        state["stag"] = spool.tile([128, NS, elem], BF16,
                                                   tag=f"stag{tag}",
                                                   name=f"stag{tag}")

                    def flush_chunk():
                        ch = state["chunk"]
                        nb = sched.chunk_sizes[ch]
                        nc.gpsimd.dma_scatter_add(
                            dst[:, 0:elem], state["stag"][:, :nb, :],
                            sc_sb[:, state["sc_off"]:state["sc_off"] + nb * 8],
                            nb * 128, nb * 128, elem, elem_step=width,
                            single_packet=False, queue_num=_q())
                        state["sc_off"] += nb * 8
                        state["chunk"] += 1

                    open_chunk()
                    for t in range(sched.ntiles):
                        G = gpool.tile([128, TC, width], BF16, tag=f"G{tag}")
                        for o in range(ops_per_tile):
                            nc.gpsimd.dma_gather(
                                G[:, o * 8:(o + 1) * 8, :], src_ap,
                                g_sb[:, (t * ops_per_tile + o) * (GOP // 16):
                                     (t * ops_per_tile + o + 1) * (GOP // 16)],
                                GOP, GOP, width, single_packet=SP,
                                queue_num=_q())
                        for (_, c0, k, B, b0, ch) in groups_by_tile.get(t, []):
                            if ch != state["chunk"]:
                                flush_chunk()
                                open_chunk()
                            bpos = b0 - ch * NS
                            _fold_reduce(nc, G, state["stag"], c0, k, B,
                                         bpos, width, elem)
                    flush_chunk()

                run_pass(schedA, gA_sb, scA_sb, nfb[:], UD,
                         elem=128 + H, width=256, tag="A")
                # edge shard projection: issued after pass A, completes
                # under the ReduceScatter window (Tensor/DVE are idle then)
                with (tc.tile_pool(name="ep", bufs=1) as epool,
                      tc.tile_pool(name="eps", bufs=4, space="PSUM") as epsp):
                    hc = epool.tile([128, BH * 128], BF16, tag="hc")
                    nc.sync.dma_start(hc[:], haTb[:])
                    for b in range(BH):
                        mm = epsp.tile([128, 128], F32, tag="mm")
                        nc.tensor.matmul(mm[:], hc[:, b * 128:(b + 1) * 128],
                                         We_sb[:], start=True, stop=True)
                        nc.vector.tensor_copy(ef_sb[:, b, :], mm[:])
                nc.gpsimd.collective_compute(
                    "ReduceScatter", ALU.add, replica_groups=replica_groups,
                    ins=[UD[0:ET2, :]], outs=[UDr[:]])

                # ---- EA' = (U/D + ef)/D on shard, then AllGather ----
                with tc.tile_pool(name="ea", bufs=3) as eap:
                    for i in range(BH):
                        r0 = i * 128
                        P = min(128, SH - r0)
                        dchunk = eap.tile([128, 256], BF16, tag="dchunk")
                        nc.sync.dma_start(dchunk[0:P, :], UDr[r0:r0 + P, :])
                        d4 = eap.tile([128, H], F32, tag="d4")
                        nc.vector.tensor_copy(d4[0:P, :], dchunk[0:P, 128:128 + H])
                        nc.vector.tensor_scalar_add(d4[0:P, :], d4[0:P, :], 1e-20)
                        inv = eap.tile([128, H], BF16, tag="inv")
                        nc.vector.reciprocal(d4[0:P, :], d4[0:P, :])
                        nc.vector.tensor_copy(inv[0:P, :], d4[0:P, :])
                        uview = _ap(dchunk[0:P, :], 0, [[32, H], [1, C]])
                        iview = _ap(inv[0:P, :], 0, [[1, H], [0, C]])
                        nc.vector.tensor_tensor(out=uview, in0=uview,
                                                in1=iview, op=ALU.mult)
                        nc.vector.tensor_tensor(
                            out=dchunk[0:P, 0:128], in0=dchunk[0:P, 0:128],
                            in1=ef_sb[0:P, i, :], op=ALU.add)
                        nc.vector.tensor_tensor(out=uview, in0=uview,
                                                in1=iview, op=ALU.mult)
                        nc.sync.dma_start(EAsh[r0:r0 + P, :],
                                          dchunk[0:P, 0:128])
                    nc.gpsimd.collective_compute(
                        "AllGather", ALU.bypass, replica_groups=replica_groups,
                        ins=[EAsh[:]], outs=[EAp[:]])
                    # zero junk edge rows [E, ET2)
                    nj = ET2 - cfg.E
                    nc.sync.dma_start(EAp[cfg.E:ET2, :], zeros_sb[0:nj, 0:128])

                # ---- pass B ----
                run_pass(schedB, gB_sb, scB_sb, EAp[:], ytmp,
                         elem=128, width=128, tag="B")

            # ---- final: y = w * ytmp + bias ----
            with tc.tile_pool(name="fin", bufs=2) as fpool:
                for ci in range(n_node_chunks):
                    a0 = ci * CHK
                    aa = min(CHK, n_a_node - a0)
                    ytc = fpool.tile([128, CHK, 128], BF16, tag="ytc")
                    nc.sync.dma_start(ytc[:, :aa, :], ytmp_v[:, a0:a0 + aa, :])
                    yst = fpool.tile([128, CHK, 128], F32, tag="yst")
                    nc.vector.tensor_tensor(
                        out=_ap(yst[:], 0, [[128, aa], [32, H], [1, C]]),
                        in0=_ap(ytc[:], 0, [[128, aa], [32, H], [1, C]]),
                        in1=_ap(w_all[:], a0 * H, [[H, aa], [1, H], [0, C]]),
                        op=ALU.mult)
                    nc.vector.tensor_tensor(
                        out=yst[:, :aa, :], in0=yst[:, :aa, :],
                        in1=bias_sb[:].unsqueeze(1).broadcast_to([128, aa, 128]),
                        op=ALU.add)
                    nc.sync.dma_start(y_v[:, a0:a0 + aa, :], yst[:, :aa, :])
    nc.compile()
    return nc


def host_inputs(cfg: Cfg, x, ha, W_node, W_edge, attn_l, bias, streams):
    import ml_dtypes
    bf16 = ml_dtypes.bfloat16
    x = np.asarray(x, np.float32)
    ha = np.asarray(ha, np.float32)
    W_node = np.asarray(W_node, np.float32)
    W_edge = np.asarray(W_edge, np.float32)
    attn_flat = np.asarray(attn_l, np.float32).reshape(-1)
    bias = np.asarray(bias, np.float32).reshape(-1)
    attn_rep = np.tile(attn_flat[None, :], (128, 1)).astype(bf16)
    bias_rep = np.tile(bias[None, :], (128, 1)).astype(np.float32)
    ha_pad = np.zeros((cfg.ET2, cfg.D), np.float32)
    ha_pad[:cfg.E] = ha
    in_maps = []
    for m in range(cfg.NC):
        xs = np.zeros((cfg.NT, cfg.D), np.float32)
        xs[:cfg.NSH] = x[m * cfg.NSH:(m + 1) * cfg.NSH]
        # edge shard columns: BH blocks of 128 starting at m*SH
        haT_shard = np.zeros((cfg.D, cfg.BH * 128), np.float32)
        blk = ha_pad[m * cfg.SH:(m + 1) * cfg.SH].T    # [D, SH]
        haT_shard[:, :cfg.SH] = blk
        st = streams[m]
        in_maps.append({
            "xTb": np.ascontiguousarray(xs.T).astype(bf16),
            "haTb": haT_shard.astype(bf16),
            "Wn": W_node.astype(bf16), "We": W_edge.astype(bf16),
            "attn": attn_rep, "bias_t": bias_rep,
            "gA_i": st["gA"], "scA_i": st["scA"],
            "gB_i": st["gB"], "scB_i": st["scB"],
        })
    return in_maps


# ======================== public entry point ========================
_CFG = Cfg()
LAST_RESULTS = None


def _install_axon_ntff_shim():
    import sys, types, ctypes, contextlib
    import concourse.bass_utils as bu
    bu.upload_artifacts = lambda d: str(d)
    try:
        from antenv.axon_hooks import get_axon_ntff_profile_hook  # noqa
        return
    except ImportError:
        pass
    so_path = "/opt/axon/libaxon_pjrt.so"
    try:
        lib = ctypes.CDLL(so_path)
    except OSError:
        return
    if not hasattr(lib, "axon_start_nrt_profile"):
        return
    lib.axon_start_nrt_profile.argtypes = [ctypes.POINTER(ctypes.c_int64),
                                           ctypes.c_size_t]
    lib.axon_start_nrt_profile.restype = ctypes.c_int64
    lib.axon_stop_nrt_profile.argtypes = [ctypes.c_char_p]
    lib.axon_stop_nrt_profile.restype = ctypes.c_int64

    @contextlib.contextmanager
    def _hook(output_dir, device_ids):
        import jax
        jax.devices()
        if device_ids:
            ids = (ctypes.c_int64 * len(device_ids))(*device_ids)
            rc = lib.axon_start_nrt_profile(ids, len(device_ids))
        else:
            rc = lib.axon_start_nrt_profile(None, 0)
        if rc != 0:
            raise RuntimeError(f"axon_start_nrt_profile rc={rc}")
        try:
            yield
        finally:
            n = lib.axon_stop_nrt_profile(str(output_dir).encode())
            print(f"ntff profile: {n} file(s) -> {output_dir}")

    mod = types.ModuleType("antenv.axon_hooks")
    mod.get_axon_ntff_profile_hook = lambda: _hook
    mod.set_axon_ntff_profile_hook = lambda h: None
    sys.modules["antenv.axon_hooks"] = mod


def kernel(**inputs) -> np.ndarray:
    from concourse.bass_utils import run_bass_kernel_spmd
    cfg = _CFG
    x = np.asarray(inputs["x"], np.float32)
    ha = np.asarray(inputs["hyperedge_attr"], np.float32)
    node_idx = np.asarray(inputs["node_idx"]).astype(np.int64)
    edge_idx = np.asarray(inputs["edge_idx"]).astype(np.int64)
    schedA, schedB, streams = build_plan(node_idx, edge_idx, cfg)
    nc = build_bass(cfg, schedA, schedB, [list(range(cfg.NC))])
    in_maps = host_inputs(cfg, x, ha, inputs["W_node"], inputs["W_edge"],
                          inputs["attn_l"], inputs["bias"], streams)
    trace = os.environ.get("GNN_TRACE", "0") == "1"
    if trace:
        _install_axon_ntff_shim()
    res = run_bass_kernel_spmd(nc, in_maps, list(range(cfg.NC)), trace=trace)
    global LAST_RESULTS
    LAST_RESULTS = res
    out = np.concatenate(
        [np.asarray(res.results[m]["y"], dtype=np.float32)[:cfg.NSH]
         for m in range(cfg.NC)], axis=0)
    return np.ascontiguousarray(out, dtype=np.float32)


# revision 22
# speedup vs baseline: 1.5051x; 1.0310x over previous
"""Hypergraph conv kernel for TRN2 (8 NeuronCores), v2.

Design (per core, incidences sharded by node id):
- phase 1: node projection nf = x@Wn, w = exp((nf*attn).sum per head);
  fused bf16 gather rows [w*nf (128ch) | w (4) | pad] -> nfb table (HBM).
  Edge projection only for this core's 1/8 edge shard (kept in SBUF).
- pass A (per edge-half): per-incidence gather of fused rows
  (single_packet 1024-idx ops), one segment-reduce per run group yields
  U|D fused, one dma_scatter_add per chunk into UD_h [EH+128, 256] bf16.
- collectives: ReduceScatter per half (overlaps pass A of the other
  half), EA' = (U/D + ef)/D on the 1/8 shard, AllGather into EAp bf16.
- pass B: per-incidence gather of EAp rows, segment-reduce per node
  run, scatter-add into ytmp bf16 (zero-init).
- final: y = w * ytmp + bias streamed over the node table (f32 out).
"""
import os
import numpy as np
from dataclasses import dataclass

import concourse.bass as bass
import concourse.mybir as mybir
import concourse.bacc as bacc
import concourse.tile as tile
import bass_rust
from concourse.library_config import mlp as mlp_library
from concourse._compat import get_trn_type, cdiv

F32 = mybir.dt.float32
BF16 = mybir.dt.bfloat16
I16 = mybir.dt.int16
AX = mybir.AxisListType
ALU = mybir.AluOpType
ACTF = mybir.ActivationFunctionType


@dataclass
class Cfg:
    N: int = 100000
    E: int = 25000
    D: int = 128      # in dim
    H: int = 4
    C: int = 32
    NC: int = 8
    TILE_COLS: int = 32
    GOP: int = 1024   # idxs per gather op (64 descs/engine, single-packet)
    NSTAGE: int = 16  # batches per scatter chunk

    @property
    def NSH(self):
        return self.N // self.NC

    @property
    def NT(self):     # node table rows (incl dummy row)
        return cdiv(self.NSH + 1, 128) * 128

    @property
    def ET2(self):    # edge table rows (excl junk)
        return cdiv(self.E, 256) * 256

    @property
    def SH(self):     # RS shard rows
        return self.ET2 // self.NC

    @property
    def BH(self):     # ef blocks
        return cdiv(self.SH, 128)

    @property
    def DUMMY_NODE(self):
        return self.NSH

    @property
    def JUNK_EDGE(self):
        return self.E


def _runs(keys):
    if len(keys) == 0:
        return (np.zeros(0, np.int64),) * 3
    change = np.flatnonzero(np.diff(keys)) + 1
    starts = np.concatenate([[0], change]).astype(np.int64)
    ends = np.concatenate([change, [len(keys)]]).astype(np.int64)
    return starts, ends - starts, keys[starts].astype(np.int64)


@dataclass
class Sched:
    batches: list          # [(k, tile, c0)]
    groups: list           # [(tile, c0, k, B, b0, chunk)]
    ntiles: int
    nchunks: int
    nbatch: int
    chunk_sizes: list


def _mk_schedule(lens_list, cfg: Cfg) -> Sched:
    # sorted-ceiling batching, common widths across cores (SPMD program)
    sorted_lens = [np.sort(np.asarray(l))[::-1] for l in lens_list]
    nbatch_total = max(cdiv(len(l), 128) for l in sorted_lens)
    batches = []
    for b in range(nbatch_total):
        w = 1
        for ls in sorted_lens:
            if b * 128 < len(ls):
                w = max(w, int(ls[b * 128]))
        batches.append(w)
    assert max(batches) <= cfg.TILE_COLS, \
        f"run length {max(batches)} > TILE_COLS"
    placed = []
    t, c = 0, 0
    for k in batches:
        if c + k > cfg.TILE_COLS:
            t += 1
            c = 0
        placed.append((k, t, c))
        c += k
    ntiles = t + 1 if placed else 1
    nbatch = len(placed)
    nchunks = cdiv(nbatch, cfg.NSTAGE)
    chunk_sizes = [min(cfg.NSTAGE, nbatch - i * cfg.NSTAGE)
                   for i in range(nchunks)]
    groups = []
    for bi, (k, t, c0) in enumerate(placed):
        ch = bi // cfg.NSTAGE
        if groups and groups[-1][0] == t and groups[-1][2] == k \
                and groups[-1][5] == ch \
                and groups[-1][1] + groups[-1][2] * groups[-1][3] == c0 \
                and groups[-1][4] + groups[-1][3] == bi:
            t0, c0g, kg, B, b0, chg = groups[-1]
            groups[-1] = (t0, c0g, kg, B + 1, b0, chg)
        else:
            groups.append((t, c0, k, 1, bi, ch))
    return Sched(placed, groups, ntiles, nchunks, nbatch, chunk_sizes)


def _wrap16(flat):
    assert len(flat) % 16 == 0
    b = flat.reshape(-1, 16).T.astype(np.int16)
    return np.tile(b, (8, 1))


def _mk_streams(sched: Sched, starts, lens, gvals, svals,
                dummy_g, junk_s, cfg: Cfg):
    """Gather stream (per 1024-idx op) + scatter stream (per chunk)."""
    TC = cfg.TILE_COLS
    g_arr = np.full((sched.ntiles, TC, 128), dummy_g, np.int64)
    s_arr = np.full((sched.nbatch, 128), junk_s, np.int64)
    order = np.argsort(-lens, kind="stable") if len(lens) else np.zeros(0, np.int64)
    for bi, (k, t, c0) in enumerate(sched.batches):
        idxs = order[bi * 128:(bi + 1) * 128]
        nr = len(idxs)
        if nr:
            st = starts[idxs]
            kr = lens[idxs]
            assert kr[0] <= k
            for kk in np.unique(kr):
                sel = np.flatnonzero(kr == kk)
                gm = gvals[st[sel][None, :] + np.arange(kk)[:, None]]
                g_arr[t, c0:c0 + kk, sel] = gm.T
            s_arr[bi, :nr] = svals[idxs]
    nops_per_tile = TC // 8
    g_idx = np.concatenate(
        [_wrap16(g_arr[t, o * 8:(o + 1) * 8].reshape(-1))
         for t in range(sched.ntiles) for o in range(nops_per_tile)], axis=1)
    sc_blocks = []
    off = 0
    for nb in sched.chunk_sizes:
        sc_blocks.append(_wrap16(s_arr[off:off + nb].reshape(-1)))
        off += nb
    sc_idx = np.concatenate(sc_blocks, axis=1)
    return g_idx, sc_idx


def build_plan(node_idx, edge_idx, cfg: Cfg):
    node_idx = np.asarray(node_idx).astype(np.int64)
    edge_idx = np.asarray(edge_idx).astype(np.int64)
    percore = []
    for m in range(cfg.NC):
        sel = np.flatnonzero(node_idx // cfg.NSH == m)
        nl = node_idx[sel] - m * cfg.NSH
        eg = edge_idx[sel]
        sA, lA, vA = _runs(eg)
        oB = np.argsort(nl, kind="stable")
        nB = nl[oB]
        eB = eg[oB]
        sB, lB, vB = _runs(nB)
        percore.append(dict(nl=nl, sA=sA, lA=lA, vA=vA,
                            eB=eB, sB=sB, lB=lB, vB=vB))
    schedA = _mk_schedule([c["lA"] for c in percore], cfg)
    schedB = _mk_schedule([c["lB"] for c in percore], cfg)
    streams = []
    for c in percore:
        st = {}
        gA, scA = _mk_streams(
            schedA, c["sA"], c["lA"],
            gvals=c["nl"], svals=c["vA"],
            dummy_g=cfg.DUMMY_NODE, junk_s=cfg.ET2, cfg=cfg)
        st["gA"] = gA
        st["scA"] = scA
        gB, scB = _mk_streams(
            schedB, c["sB"], c["lB"],
            gvals=c["eB"], svals=c["vB"],
            dummy_g=cfg.JUNK_EDGE, junk_s=cfg.DUMMY_NODE, cfg=cfg)
        st["gB"] = gB
        st["scB"] = scB
        streams.append(st)
    return schedA, schedB, streams


def _ap(t_ap, off, dims):
    base = t_ap
    part = base.ap[0]
    return bass_rust.AP(base.tensor, base.offset + off, [part] + dims)


def _collective_on(nc, eng, kind, op, replica_groups, in_ap, out_ap):
    """collective_compute issued from an arbitrary engine so it does not
    block the Pool (gpsimd) queue. Mirrors BassGpSimd.collective_compute
    for the internal-DRAM cases used here."""
    from concourse.bass import filter_and_check_groups
    nc.has_collectives = True
    rg = filter_and_check_groups(nc.num_devices, replica_groups)
    return eng.add_instruction(
        mybir.InstCollectiveCompute(
            name=f"I-{nc.next_id()}",
            kind=kind,
            op=op,
            replica_groups=rg,
            ins=[eng.lower_ap(in_ap)],
            outs=[eng.lower_ap(out_ap)],
            unique_tensors="No",
            cc_dim="Partition",
        )
    )


def _fold_reduce(nc, G, stag, c0, k, B, bpos, width, elem):
    """Segment sum over k columns per run via fold-tree tensor_tensor adds
    (innermost stride 1 -> DVE 2x perf mode), final fold/copy into stag."""
    W = width
    kk = k
    while kk > 2:
        h = kk // 2
        rem = kk - h
        nc.vector.tensor_tensor(
            out=_ap(G[:], c0 * W, [[k * W, B], [W, h], [1, elem]]),
            in0=_ap(G[:], c0 * W, [[k * W, B], [W, h], [1, elem]]),
            in1=_ap(G[:], (c0 + rem) * W, [[k * W, B], [W, h], [1, elem]]),
            op=ALU.add)
        kk = rem
    if kk == 2:
        nc.vector.tensor_tensor(
            out=_ap(stag[:], bpos * elem, [[elem, B], [elem, 1], [1, elem]]),
            in0=_ap(G[:], c0 * W, [[k * W, B], [W, 1], [1, elem]]),
            in1=_ap(G[:], (c0 + 1) * W, [[k * W, B], [W, 1], [1, elem]]),
            op=ALU.add)
    else:
        nc.vector.tensor_copy(
            _ap(stag[:], bpos * elem, [[elem, B], [1, elem]]),
            _ap(G[:], c0 * W, [[k * W, B], [1, elem]]))


def build_bass(cfg: Cfg, schedA, schedB, replica_groups):
    SP = os.environ.get("GNN_SP", "1") == "1"
    TC, NS, GOP = cfg.TILE_COLS, cfg.NSTAGE, cfg.GOP
    H, C = cfg.H, cfg.C
    NT, ET2, SH, BH = cfg.NT, cfg.ET2, cfg.SH, cfg.BH
    NSH = cfg.NSH
    n_a_node = NT // 128
    CHK = 16
    n_node_chunks = cdiv(n_a_node, CHK)
    ops_per_tile = TC // 8

    nc = bacc.Bacc(get_trn_type() or "TRN2", target_bir_lowering=False,
                   debug=False, num_swdge_queues=4)
    _qrr = [0]

    def _q():
        _qrr[0] += 1
        return _qrr[0] % 4

    # ---- I/O ----
    xTb = nc.dram_tensor("xTb", [128, NT], BF16, kind="ExternalInput")
    haTb = nc.dram_tensor("haTb", [128, BH * 128], BF16, kind="ExternalInput")
    Wn = nc.dram_tensor("Wn", [128, H * C], BF16, kind="ExternalInput")
    We = nc.dram_tensor("We", [128, H * C], BF16, kind="ExternalInput")
    attn = nc.dram_tensor("attn", [128, H * C], BF16, kind="ExternalInput")
    bias_t = nc.dram_tensor("bias_t", [128, H * C], BF16, kind="ExternalInput")
    gA_i = nc.dram_tensor("gA_i", [128, schedA.ntiles * ops_per_tile * (GOP // 16)],
                          I16, kind="ExternalInput")
    scA_i = nc.dram_tensor("scA_i", [128, schedA.nbatch * 8], I16,
                           kind="ExternalInput")
    gB_i = nc.dram_tensor("gB_i", [128, schedB.ntiles * ops_per_tile * (GOP // 16)],
                          I16, kind="ExternalInput")
    scB_i = nc.dram_tensor("scB_i", [128, schedB.nbatch * 8], I16,
                           kind="ExternalInput")
    y = nc.dram_tensor("y", [NT, H * C], BF16, kind="ExternalOutput")

    # ---- internal DRAM ----
    nfb = nc.dram_tensor("nfb", [NT, 256], BF16)       # [w*nf | w*4 | pad]
    UD = nc.dram_tensor("UD", [ET2 + 128, 256], BF16)
    UDr = nc.dram_tensor("UDr", [SH, 256], BF16)
    EAsh = nc.dram_tensor("EAsh", [SH, 128], BF16)
    EAp = nc.dram_tensor("EAp", [ET2, 128], BF16, addr_space="Shared")
    ytmp = nc.dram_tensor("ytmp", [NT, 128], BF16)

    nfb_v = nfb[:].rearrange("(a p) c -> p a c", p=128)
    ud_v = UD[:].rearrange("(a p) c -> p a c", p=128)
    ytmp_v = ytmp[:].rearrange("(a p) c -> p a c", p=128)
    y_v = y[:].rearrange("(a p) c -> p a c", p=128)

    with tile.TileContext(nc) as tc, \
            nc.allow_low_precision(reason="2e-2 tolerance, bf16 throughout"), \
            tc.tile_pool(name="const", bufs=1) as cpool:
        if True:
            nc.gpsimd.load_library(mlp_library)
            Wn_sb = cpool.tile([128, H * C], BF16)
            We_sb = cpool.tile([128, H * C], BF16)
            attn_sb = cpool.tile([128, H * C], BF16)
            bias_sb = cpool.tile([128, H * C], BF16)
            zeros_sb = cpool.tile([128, 4096], BF16)
            w_all = cpool.tile([128, n_a_node, H], BF16)
            ef_sb = cpool.tile([128, BH, 128], BF16)
            nc.sync.dma_start(Wn_sb[:], Wn[:])
            nc.sync.dma_start(We_sb[:], We[:])
            nc.sync.dma_start(attn_sb[:], attn[:])
            nc.sync.dma_start(bias_sb[:], bias_t[:])
            nc.gpsimd.memset(zeros_sb[:], 0.0)

            # ---- init: UD=0, ytmp=0 ----
            zv16 = zeros_sb[:].rearrange("p (a c) -> p a c", c=256)  # [128,16,256]
            na = (ET2 + 128) // 128
            for a0 in range(0, na, 16):
                aa = min(16, na - a0)
                nc.sync.dma_start(ud_v[:, a0:a0 + aa, :], zv16[:, :aa, :])
            zv8 = zeros_sb[:].rearrange("p (a c) -> p a c", c=128)   # [128,32,128]
            for a0 in range(0, n_a_node, 32):
                aa = min(32, n_a_node - a0)
                nc.sync.dma_start(ytmp_v[:, a0:a0 + aa, :], zv8[:, :aa, :])

            # ---- phase 1: projections ----
            with (tc.tile_pool(name="p1", bufs=2) as p1,
                  tc.tile_pool(name="p1s", bufs=2) as p1s,
                  tc.tile_pool(name="ps", bufs=4, space="PSUM") as psp):
                # node side: fused pre-weighted rows
                for ci in range(n_node_chunks):
                    a0 = ci * CHK
                    aa = min(CHK, n_a_node - a0)
                    xc = p1.tile([128, CHK * 128], BF16, tag="xc")
                    nc.sync.dma_start(xc[:, :aa * 128], xTb[:, a0 * 128:(a0 + aa) * 128])
                    nfst = p1s.tile([128, CHK, 256], BF16, tag="nfst")
                    for i0 in range(0, aa, 4):
                        ii = min(4, aa - i0)
                        mm = psp.tile([128, 4, 128], F32, tag="mm")
                        for j in range(ii):
                            nc.tensor.matmul(
                                mm[:, j, :],
                                xc[:, (i0 + j) * 128:(i0 + j + 1) * 128],
                                Wn_sb[:], start=True, stop=True)
                        nc.vector.tensor_copy(
                            _ap(nfst[:], i0 * 256, [[256, ii], [1, 128]]),
                            mm[:, :ii, :])
                    # attn dot -> praw -> w = exp(praw)
                    tmp1 = p1s.tile([128, CHK, 128], BF16, tag="tmp1")
                    nc.vector.tensor_tensor(
                        out=tmp1[:, :aa, :].rearrange("p a (h c) -> p a h c", h=H),
                        in0=nfst[:, :aa, 0:128].rearrange("p a (h c) -> p a h c", h=H),
                        in1=attn_sb[:].rearrange("p (h c) -> p h c", h=H)
                            .unsqueeze(1).broadcast_to([128, aa, H, C]),
                        op=ALU.mult)
                    praw = p1s.tile([128, CHK, H], F32, tag="praw")
                    nc.vector.tensor_reduce(
                        out=praw[:, :aa, :],
                        in_=tmp1[:, :aa, :].rearrange("p a (h c) -> p a h c", h=H),
                        axis=AX.X, op=ALU.add)
                    west = p1s.tile([128, CHK, H], BF16, tag="west")
                    nc.scalar.activation(west[:, :aa, :], praw[:, :aa, :], ACTF.Exp)
                    nc.vector.tensor_copy(w_all[:, a0:a0 + aa, :], west[:, :aa, :])
                    # fused row: [w*nf | w | pad124]
                    nc.vector.tensor_tensor(
                        out=_ap(nfst[:], 0, [[256, aa], [32, H], [1, C]]),
                        in0=_ap(nfst[:], 0, [[256, aa], [32, H], [1, C]]),
                        in1=_ap(west[:], 0, [[H, aa], [1, H], [0, C]]),
                        op=ALU.mult)
                    nc.vector.tensor_copy(nfst[:, :aa, 128:128 + H], west[:, :aa, :])
                    nc.gpsimd.memset(
                        _ap(nfst[:], 128 + H, [[256, aa], [1, 128 - H]]), 0.0)
                    nc.sync.dma_start(nfb_v[:, a0:a0 + aa, :], nfst[:, :aa, :])
                # zero dummy node rows (NSH..NT): all-zero fused rows
                nc.sync.dma_start(nfb[NSH:NT, :], zeros_sb[0:NT - NSH, 0:256])


            # ---- pass A (both halves) + per-half ReduceScatter ----
            with (tc.tile_pool(name="gA", bufs=3) as gpool,
                  tc.tile_pool(name="stA", bufs=3) as spool,
                  tc.tile_pool(name="idxA", bufs=1) as ipool):
                gA_sb = ipool.tile([128, gA_i.shape[1]], I16)
                scA_sb = ipool.tile([128, scA_i.shape[1]], I16)
                gB_sb = ipool.tile([128, gB_i.shape[1]], I16)
                scB_sb = ipool.tile([128, scB_i.shape[1]], I16)
                nc.sync.dma_start(gA_sb[:], gA_i[:])
                nc.sync.dma_start(scA_sb[:], scA_i[:])
                nc.sync.dma_start(gB_sb[:], gB_i[:])
                nc.sync.dma_start(scB_sb[:], scB_i[:])

                def run_pass(sched, g_sb, sc_sb, src_ap, dst, elem, width,
                             tag):
                    groups_by_tile = {}
                    for g in sched.groups:
                        groups_by_tile.setdefault(g[0], []).append(g)
                    state = {"chunk": 0, "sc_off": 0, "stag": None}

                    def open_chunk():
                        state["stag"] = spool.tile([128, NS, elem], BF16,
                                                   tag=f"stag{tag}",
                                                   name=f"stag{tag}")

                    def flush_chunk():
                        ch = state["chunk"]
                        nb = sched.chunk_sizes[ch]
                        nc.gpsimd.dma_scatter_add(
                            dst[:, 0:elem], state["stag"][:, :nb, :],
                            sc_sb[:, state["sc_off"]:state["sc_off"] + nb * 8],
                            nb * 128, nb * 128, elem, elem_step=width,
                            single_packet=False, queue_num=_q())
                        state["sc_off"] += nb * 8
                        state["chunk"] += 1

                    open_chunk()
                    for t in range(sched.ntiles):
                        G = gpool.tile([128, TC, width], BF16, tag=f"G{tag}")
                        for o in range(ops_per_tile):
                            nc.gpsimd.dma_gather(
                                G[:, o * 8:(o + 1) * 8, :], src_ap,
                                g_sb[:, (t * ops_per_tile + o) * (GOP // 16):
                                     (t * ops_per_tile + o + 1) * (GOP // 16)],
                                GOP, GOP, width, single_packet=SP,
                                queue_num=_q())
                        for (_, c0, k, B, b0, ch) in groups_by_tile.get(t, []):
                            if ch != state["chunk"]:
                                flush_chunk()
                                open_chunk()
                            bpos = b0 - ch * NS
                            _fold_reduce(nc, G, state["stag"], c0, k, B,
                                         bpos, width, elem)
                    flush_chunk()

                run_pass(schedA, gA_sb, scA_sb, nfb[:], UD,
                         elem=128 + H, width=256, tag="A")
                # edge shard projection: issued after pass A, completes
                # under the ReduceScatter window (Tensor/DVE are idle then)
                with (tc.tile_pool(name="ep", bufs=1) as epool,
                      tc.tile_pool(name="eps", bufs=4, space="PSUM") as epsp):
                    hc = epool.tile([128, BH * 128], BF16, tag="hc")
                    nc.sync.dma_start(hc[:], haTb[:])
                    for b0 in range(0, BH, 4):
                        ii = min(4, BH - b0)
                        mm = epsp.tile([128, 4, 128], F32, tag="mm")
                        for j in range(ii):
                            nc.tensor.matmul(
                                mm[:, j, :],
                                hc[:, (b0 + j) * 128:(b0 + j + 1) * 128],
                                We_sb[:], start=True, stop=True)
                        nc.vector.tensor_copy(ef_sb[:, b0:b0 + ii, :],
                                              mm[:, :ii, :])
                nc.gpsimd.collective_compute(
                    "ReduceScatter", ALU.add, replica_groups=replica_groups,
                    ins=[UD[0:ET2, :]], outs=[UDr[:]])

                # ---- EA' = (U/D + ef)/D on shard (one shot), AllGather ----
                with tc.tile_pool(name="ea", bufs=1) as eap:
                    # shard as [128, BH, 256]: row r of UDr at
                    # (partition r%128, block r//128); tail block partial.
                    nfull = SH // 128
                    Ptail = SH - nfull * 128
                    dchunk = eap.tile([128, BH, 256], BF16, tag="dchunk")
                    if Ptail:
                        nc.gpsimd.memset(dchunk[:, nfull, :], 0.0)
                    if nfull:
                        nc.sync.dma_start(
                            dchunk[:, :nfull, :],
                            UDr[0:nfull * 128, :].rearrange(
                                "(a p) c -> p a c", p=128))
                    if Ptail:
                        nc.sync.dma_start(dchunk[0:Ptail, nfull, :],
                                          UDr[nfull * 128:SH, :])
                    d4 = eap.tile([128, BH, H], F32, tag="d4")
                    nc.vector.tensor_copy(d4[:], dchunk[:, :, 128:128 + H])
                    nc.vector.tensor_scalar_add(d4[:], d4[:], 1e-20)
                    inv = eap.tile([128, BH, H], BF16, tag="inv")
                    nc.vector.reciprocal(d4[:], d4[:])
                    nc.vector.tensor_copy(inv[:], d4[:])
                    uview = _ap(dchunk[:], 0, [[256, BH], [32, H], [1, C]])
                    iview = _ap(inv[:], 0, [[H, BH], [1, H], [0, C]])
                    nc.vector.tensor_tensor(out=uview, in0=uview,
                                            in1=iview, op=ALU.mult)
                    nc.vector.tensor_tensor(
                        out=_ap(dchunk[:], 0, [[256, BH], [1, 128]]),
                        in0=_ap(dchunk[:], 0, [[256, BH], [1, 128]]),
                        in1=ef_sb[:],
                        op=ALU.add)
                    nc.vector.tensor_tensor(out=uview, in0=uview,
                                            in1=iview, op=ALU.mult)
                    if nfull:
                        nc.sync.dma_start(
                            EAsh[0:nfull * 128, :].rearrange(
                                "(a p) c -> p a c", p=128),
                            dchunk[:, :nfull, 0:128])
                    if Ptail:
                        nc.sync.dma_start(EAsh[nfull * 128:SH, :],
                                          dchunk[0:Ptail, nfull, 0:128])
                    nc.gpsimd.collective_compute(
                        "AllGather", ALU.bypass, replica_groups=replica_groups,
                        ins=[EAsh[:]], outs=[EAp[:]])
                    # zero junk edge rows [E, ET2)
                    nj = ET2 - cfg.E
                    nc.sync.dma_start(EAp[cfg.E:ET2, :], zeros_sb[0:nj, 0:128])

                # ---- pass B ----
                run_pass(schedB, gB_sb, scB_sb, EAp[:], ytmp,
                         elem=128, width=128, tag="B")

            # ---- final: y = w * ytmp + bias ----
            with tc.tile_pool(name="fin", bufs=2) as fpool:
                for ci in range(n_node_chunks):
                    a0 = ci * CHK
                    aa = min(CHK, n_a_node - a0)
                    ytc = fpool.tile([128, CHK, 128], BF16, tag="ytc")
                    nc.sync.dma_start(ytc[:, :aa, :], ytmp_v[:, a0:a0 + aa, :])
                    yst = fpool.tile([128, CHK, 128], BF16, tag="yst")
                    nc.vector.tensor_tensor(
                        out=_ap(yst[:], 0, [[128, aa], [32, H], [1, C]]),
                        in0=_ap(ytc[:], 0, [[128, aa], [32, H], [1, C]]),
                        in1=_ap(w_all[:], a0 * H, [[H, aa], [1, H], [0, C]]),
                        op=ALU.mult)
                    nc.vector.tensor_tensor(
                        out=yst[:, :aa, :], in0=yst[:, :aa, :],
                        in1=bias_sb[:].unsqueeze(1).broadcast_to([128, aa, 128]),
                        op=ALU.add)
                    nc.sync.dma_start(y_v[:, a0:a0 + aa, :], yst[:, :aa, :])
    nc.compile()
    return nc


def host_inputs(cfg: Cfg, x, ha, W_node, W_edge, attn_l, bias, streams):
    import ml_dtypes
    bf16 = ml_dtypes.bfloat16
    x = np.asarray(x, np.float32)
    ha = np.asarray(ha, np.float32)
    W_node = np.asarray(W_node, np.float32)
    W_edge = np.asarray(W_edge, np.float32)
    attn_flat = np.asarray(attn_l, np.float32).reshape(-1)
    bias = np.asarray(bias, np.float32).reshape(-1)
    attn_rep = np.tile(attn_flat[None, :], (128, 1)).astype(bf16)
    bias_rep = np.tile(bias[None, :], (128, 1)).astype(bf16)
    ha_pad = np.zeros((cfg.ET2, cfg.D), np.float32)
    ha_pad[:cfg.E] = ha
    in_maps = []
    for m in range(cfg.NC):
        xs = np.zeros((cfg.NT, cfg.D), np.float32)
        xs[:cfg.NSH] = x[m * cfg.NSH:(m + 1) * cfg.NSH]
        # edge shard columns: BH blocks of 128 starting at m*SH
        haT_shard = np.zeros((cfg.D, cfg.BH * 128), np.float32)
        blk = ha_pad[m * cfg.SH:(m + 1) * cfg.SH].T    # [D, SH]
        haT_shard[:, :cfg.SH] = blk
        st = streams[m]
        in_maps.append({
            "xTb": np.ascontiguousarray(xs.T).astype(bf16),
            "haTb": haT_shard.astype(bf16),
            "Wn": W_node.astype(bf16), "We": W_edge.astype(bf16),
            "attn": attn_rep, "bias_t": bias_rep,
            "gA_i": st["gA"], "scA_i": st["scA"],
            "gB_i": st["gB"], "scB_i": st["scB"],
        })
    return in_maps


# ======================== public entry point ========================
_CFG = Cfg()
LAST_RESULTS = None


def _install_axon_ntff_shim():
    import sys, types, ctypes, contextlib
    import concourse.bass_utils as bu
    bu.upload_artifacts = lambda d: str(d)
    try:
        from antenv.axon_hooks import get_axon_ntff_profile_hook  # noqa
        return
    except ImportError:
        pass
    so_path = "/opt/axon/libaxon_pjrt.so"
    try:
        lib = ctypes.CDLL(so_path)
    except OSError:
        return
    if not hasattr(lib, "axon_start_nrt_profile"):
        return
    lib.axon_start_nrt_profile.argtypes = [ctypes.POINTER(ctypes.c_int64),
                                           ctypes.c_size_t]
    lib.axon_start_nrt_profile.restype = ctypes.c_int64
    lib.axon_stop_nrt_profile.argtypes = [ctypes.c_char_p]
    lib.axon_stop_nrt_profile.restype = ctypes.c_int64

    @contextlib.contextmanager
    def _hook(output_dir, device_ids):
        import jax
        jax.devices()
        if device_ids:
            ids = (ctypes.c_int64 * len(device_ids))(*device_ids)
            rc = lib.axon_start_nrt_profile(ids, len(device_ids))
        else:
            rc = lib.axon_start_nrt_profile(None, 0)
        if rc != 0:
            raise RuntimeError(f"axon_start_nrt_profile rc={rc}")
        try:
            yield
        finally:
            n = lib.axon_stop_nrt_profile(str(output_dir).encode())
            print(f"ntff profile: {n} file(s) -> {output_dir}")

    mod = types.ModuleType("antenv.axon_hooks")
    mod.get_axon_ntff_profile_hook = lambda: _hook
    mod.set_axon_ntff_profile_hook = lambda h: None
    sys.modules["antenv.axon_hooks"] = mod


def kernel(**inputs) -> np.ndarray:
    from concourse.bass_utils import run_bass_kernel_spmd
    cfg = _CFG
    x = np.asarray(inputs["x"], np.float32)
    ha = np.asarray(inputs["hyperedge_attr"], np.float32)
    node_idx = np.asarray(inputs["node_idx"]).astype(np.int64)
    edge_idx = np.asarray(inputs["edge_idx"]).astype(np.int64)
    schedA, schedB, streams = build_plan(node_idx, edge_idx, cfg)
    nc = build_bass(cfg, schedA, schedB, [list(range(cfg.NC))])
    in_maps = host_inputs(cfg, x, ha, inputs["W_node"], inputs["W_edge"],
                          inputs["attn_l"], inputs["bias"], streams)
    trace = os.environ.get("GNN_TRACE", "0") == "1"
    if trace:
        _install_axon_ntff_shim()
    res = run_bass_kernel_spmd(nc, in_maps, list(range(cfg.NC)), trace=trace)
    global LAST_RESULTS
    LAST_RESULTS = res
    out = np.concatenate(
        [np.asarray(res.results[m]["y"], dtype=np.float32)[:cfg.NSH]
         for m in range(cfg.NC)], axis=0)
    return np.ascontiguousarray(out, dtype=np.float32)
